# revision 1
# baseline (speedup 1.0000x reference)
"""ConvLSTM net (nn_Net_50354196578736) Trainium2 Bass kernel.

Data-parallel over batch: B=8 -> 1 sample per NeuronCore, 8 cores, no
collectives. Per core:
  clstm1 (T=32, 33->128ch, 3x3 SAME on 8x256) -> maxpool3d 2x2x2
  clstm2 (T=16, 80->192ch, 3x3 SAME on 4x128) -> maxpool3d 2x2x2
  reshape -> conv3 (256,48,3,64) VALID + ELU -> conv4 1x1 + ELU -> conv5 1x1

Conv-as-matmul: channels on partitions, zero-padded spatial planes on the
free dim, fp32 PSUM accumulation over shifted-view matmuls, bf16 datapath.

clstm1 K-stacking: the hidden state h (32ch) is kept in 4 partition
quadrants of the recurrent input buffer - quadrant 0 unshifted plus three
spatially shifted replicas (+1 col, +1 row, +1 row+1 col) built by
background SBUF->SBUF DMAs. Kernel offsets whose spatial deltas match the
replica shifts then stack on the contraction axis, collapsing the 9-offset
3x3 conv to 5 matmul passes: one K=128 (offsets (-1,-1),(-1,0),(0,-1),
(0,0)), one K=64 ((1,-1),(1,0)), three K=32. The x-channel contribution is
host-precomputed as a K=9 im2col and folded in as one more accumulating
matmul. (True tile_position row-tiling was probed and hard-faults when
concurrent row tiles accumulate into one PSUM bank.)

Gate math per step: z rows ordered [i,f,o,g]; one sigmoid scan over
[i,f,o]; tanh(g) straight from PSUM partition-shifted into the [tg; c]
pair tile; one paired tensor_tensor makes [sig(i)*tg; sig(f)*c]; the pair
sum c = m1+m2 runs on the PE via a stacked-identity matmul; tanh(c) lands
partition-shifted next to sig(o) for the h product, which writes the next
step's padded conv input directly.

Partition-alignment rules (verified empirically): ops with a PSUM input
may shift partitions freely; two-SBUF-input tensor_tensor needs equal
input bases (output base free); single-SBUF-input ops shift freely;
TensorCopy/Memset need 32-aligned bases.

_split_waits: this walrus build accepts only one embedded sync wait per
instruction; the pass hoists extra waits into standalone EventSemaphore
ops on the same engine. All DMAs use the single SWDGE queue for the same
reason. Host-side numpy does all weight permutation/padding/packing.
"""

import numpy as np

B, T, H, W = 8, 32, 8, 256
F1, F2, F3, F4, NN = 32, 48, 256, 128, 88
N_CORES = 8

PH1, PW1 = 10, 260   # padded layer1 plane; valid (y,x) at (y+1, x+2)
PH2, PW2 = 6, 132    # padded layer2 plane (4x128 maps)
SP1 = H * W          # 2048
SP2 = 4 * 128        # 512

_CACHE = {}


def _build_program():
    import concourse.bass as bass
    import concourse.mybir as mybir
    from concourse.tile import TileContext

    dt = mybir.dt
    AF = mybir.ActivationFunctionType
    OP = mybir.AluOpType
    BF, FP = dt.bfloat16, dt.float32

    nc = bass.Bass(trn_type="TRN2", target_bir_lowering=True, use_seq_codegen=True)

    xim_d = nc.dram_tensor("xim", [9, T * 2048], BF, kind="ExternalInput")
    w1_d = nc.dram_tensor("w1r", [128, 6 * 128], BF, kind="ExternalInput")
    w2_d = nc.dram_tensor("w2r", [96, 9 * 256], BF, kind="ExternalInput")
    w3_d = nc.dram_tensor("w3r", [128, 3 * 64 * 256], BF, kind="ExternalInput")
    w4_d = nc.dram_tensor("w4r", [128, 2 * 128], FP, kind="ExternalInput")
    cpf_d = nc.dram_tensor("cpf", [128, 368], FP, kind="ExternalInput")
    cpb_d = nc.dram_tensor("cpb", [128, 256], BF, kind="ExternalInput")
    out_d = nc.dram_tensor("out", [88, 14], FP, kind="ExternalOutput")

    with TileContext(nc) as tc:
        with tc.tile_pool(name="persist", bufs=1) as pp:
            W1 = pp.tile([128, 6, 128], BF, tag="W1")
            W2 = pp.tile([96, 9, 256], BF, tag="W2")
            W4 = pp.tile([128, 2, 128], FP, tag="W4")
            CPF = pp.tile([128, 368], FP, tag="CPF")
            CPB = pp.tile([128, 256], BF, tag="CPB")
            B1 = CPF[:, 0:1]
            B2A = CPF[:, 1:2]
            B2B = CPF[:, 2:3]
            B4 = CPF[:, 3:4]
            B5 = CPF[0:88, 4:5]
            IDT = CPF[0:14, 8:22]
            B3R = CPF[0:14, 22:278]
            W5 = CPF[:, 280:368]
            W1X = CPB[0:9, 0:128]
            IP1 = CPB[0:64, 128:160]
            IP2 = CPB[:, 160:224]
            IDTB = CPB[0:14, 224:238]
            INb = [pp.tile([128, PH1, PW1], BF, tag=f"IN{k}", name=f"IN{k}")
                   for k in range(2)]
            IN2b = [pp.tile([96, PH2, PW2], BF, tag=f"IN2{k}", name=f"IN2{k}")
                    for k in range(2)]
            TGC1 = pp.tile([64, SP1], BF, tag="TGC1")    # [tg ; c]
            TGC2 = pp.tile([128, SP2], BF, tag="TGC2")   # [c2,-,tg2,-]
            XP2 = pp.tile([32, 16, 512], BF, tag="XP2")
            PL2R = pp.tile([128, 16, 64], BF, tag="PL2R")

            dma = nc.gpsimd.dma_start
            dma(out=W1.rearrange("p a b -> p (a b)"), in_=w1_d[:, :])
            dma(out=W2.rearrange("p a b -> p (a b)"), in_=w2_d[:, :])
            dma(out=W4.rearrange("p a b -> p (a b)"), in_=w4_d[:, :])
            dma(out=CPF[:, :], in_=cpf_d[:, :])
            dma(out=CPB[:, :], in_=cpb_d[:, :])

            for k in range(2):
                nc.vector.memset(INb[k].rearrange("p a b -> p (a b)"), 0.0)
                nc.vector.memset(IN2b[k].rearrange("p a b -> p (a b)"), 0.0)
            nc.vector.memset(TGC1[:, :], 0.0)
            nc.vector.memset(TGC2[:, :], 0.0)

            # ============================= clstm1, 32 steps x 2 half-planes
            with (tc.tile_pool(name="psum1", bufs=2, space="PSUM") as ps1,
                  tc.tile_pool(name="ximp", bufs=2) as xp,
                  tc.tile_pool(name="gates1", bufs=3) as g1):
                S = g1.tile([128, SP1], BF, tag="S1", bufs=1)
                TC = g1.tile([96, SP1], BF, tag="TC", bufs=1)
                # preheat: absorb init-DMA sem into each engine's clock so
                # steady-state instructions carry <=2 sync waits
                PHP = ps1.tile([2, 4], FP, tag="Z1")
                nc.tensor.matmul(PHP[:, :], CPB[0:9, 0:2], CPB[0:9, 0:4],
                                 start=True, stop=True)
                nc.scalar.copy(S[0:2, 0:2], CPF[0:2, 0:2])
                nc.vector.tensor_copy(TGC1[0:2, 0:2], CPF[0:2, 0:2])
                for t in range(T):
                    if t % 2 == 0:
                        XIMc = xp.tile([9, 2, 2, 1024], BF, tag="XIMc",
                                       name="XIMc")
                        dma(out=XIMc.rearrange("p a b c -> p (a b c)"),
                            in_=xim_d[:, 2048 * t:2048 * (t + 2)])
                    cur, nxt = INb[t % 2], INb[(t + 1) % 2]
                    for hf in range(2):
                        hs = slice(1024 * hf, 1024 * (hf + 1))
                        Z = ps1.tile([128, 4, 256], FP, tag="Z1")
                        Zq = Z.rearrange("p a b -> p (a b)")
                        for q in range(2):
                            nc.tensor.matmul(
                                Zq[:, 512 * q:512 * (q + 1)],
                                W1X[:, :],
                                XIMc[:, t % 2, hf, 512 * q:512 * (q + 1)],
                                start=True, stop=False)
                        groups = ((0, 128, -1, -1), (1, 64, 1, -1),
                                  (2, 32, -1, 1), (3, 32, 0, 1),
                                  (4, 32, 1, 1))
                        for y in range(4):
                            yy = 4 * hf + y
                            for gi, (slot, K, dy, dx) in enumerate(groups):
                                nc.tensor.matmul(
                                    Z[:, y, :],
                                    W1[0:K, slot, :],
                                    cur[0:K, yy + 1 + dy, 2 + dx:2 + dx + 256],
                                    start=False, stop=(gi == 4))
                        Zf = Z.rearrange("p a b -> p (a b)")
                        nc.scalar.activation(S[0:96, hs], Zf[0:96, :], AF.Sigmoid,
                                             bias=B1[0:96, 0:1])
                        nc.scalar.activation(TGC1[0:32, hs], Zf[96:128, :],
                                             AF.Tanh, bias=B1[96:128, 0:1])
                        P2 = g1.tile([64, 1024], BF, tag="P2")
                        nc.vector.tensor_tensor(P2[:, :], S[0:64, hs],
                                                TGC1[:, hs], OP.mult)
                        ZC = ps1.tile([32, 1024], FP, tag="ZC")
                        for q in range(2):
                            nc.tensor.matmul(ZC[:, 512 * q:512 * (q + 1)],
                                             IP1[:, :],
                                             P2[:, 512 * q:512 * (q + 1)],
                                             start=True, stop=True)
                        nc.vector.tensor_copy(TGC1[32:64, hs], ZC[:, :])
                        nc.scalar.activation(TC[64:96, hs], ZC[:, :], AF.Tanh)
                        hview = nxt[0:32, 1 + 4 * hf:5 + 4 * hf, 2:258]
                        nc.vector.tensor_tensor(
                            hview,
                            S[64:96, hs].rearrange("p (a b) -> p a b", b=256),
                            TC[64:96, hs].rearrange("p (a b) -> p a b", b=256),
                            OP.mult)
                        r0, r1 = 1 + 4 * hf, 5 + 4 * hf
                        dma(out=nxt[32:64, r0:r1, 1:257], in_=hview)
                        dma(out=nxt[64:96, r0 - 1:r1 - 1, 2:258], in_=hview)
                        dma(out=nxt[96:128, r0 - 1:r1 - 1, 1:257], in_=hview)
                    if t % 2 == 1:
                        k = t // 2
                        PA = g1.tile([32, 8, 256], BF, tag="PA")
                        nc.vector.tensor_tensor(
                            PA[:, :, :], cur[0:32, 1:9, 2:258],
                            nxt[0:32, 1:9, 2:258], OP.max)
                        PAv = PA.rearrange("p a (b c) -> p a b c", c=2)
                        PX = g1.tile([32, 8, 128], BF, tag="PX")
                        nc.vector.tensor_tensor(
                            PX[:, :, :], PAv[:, :, :, 0], PAv[:, :, :, 1],
                            OP.max)
                        PXv = PX.rearrange("p (a c) b -> p a c b", c=2)
                        XPv = XP2.rearrange("p a (h w) -> p a h w", w=128)
                        nc.vector.tensor_tensor(
                            XPv[:, k, :, :],
                            PXv[:, :, 0, :], PXv[:, :, 1, :], OP.max)

            # ================================================ clstm2, 16 steps
            W3 = pp.tile([128, 3, 64, 256], BF, tag="W3")
            dma(out=W3.rearrange("p a b c -> p (a b c)"), in_=w3_d[:, :])
            with (tc.tile_pool(name="psum2", bufs=2, space="PSUM") as ps2,
                  tc.tile_pool(name="gates2", bufs=3) as g2):
                for t in range(16):
                    cur, nxt = IN2b[t % 2], IN2b[(t + 1) % 2]
                    nc.vector.tensor_copy(
                        cur[64:96, 1:5, 2:130],
                        XP2[:, t, :].rearrange("p (a b) -> p a b", b=128))
                    ZA = ps2.tile([128, SP2], FP, tag="ZA")
                    ZB = ps2.tile([128, SP2], FP, tag="ZB")
                    for zt, c0 in ((ZA, 0), (ZB, 128)):
                        for off in range(9):
                            dy, dx = off // 3 - 1, off % 3 - 1
                            rhs = cur[:, 1 + dy:5 + dy, 2 + dx:2 + dx + 128]
                            nc.tensor.matmul(zt[:, :], W2[:, off, c0:c0 + 128],
                                             rhs, start=(off == 0),
                                             stop=(off == 8))
                    # ZA rows [f(0:48) - i(64:112) -]; ZB [o(0:48) - g(64:112) -]
                    S2 = g2.tile([128, SP2], BF, tag="S2")
                    SO2 = g2.tile([64, SP2], BF, tag="SO2")
                    nc.scalar.activation(S2[:, :], ZA[:, :], AF.Sigmoid,
                                         bias=B2A[:, 0:1])
                    nc.scalar.activation(SO2[:, :], ZB[0:64, :], AF.Sigmoid,
                                         bias=B2B[0:64, 0:1])
                    nc.scalar.activation(TGC2[64:128, :], ZB[64:128, :],
                                         AF.Tanh, bias=B2B[64:128, 0:1])
                    P22 = g2.tile([128, SP2], BF, tag="P22")
                    nc.vector.tensor_tensor(P22[:, :], S2[:, :], TGC2[:, :],
                                            OP.mult)
                    ZC2 = ps2.tile([64, SP2], FP, tag="ZC2")
                    nc.tensor.matmul(ZC2[:, :], IP2[:, :], P22[:, :],
                                     start=True, stop=True)
                    nc.vector.tensor_copy(TGC2[0:64, :], ZC2[:, :])
                    TC2 = g2.tile([64, SP2], BF, tag="TC2")
                    nc.scalar.activation(TC2[:, :], ZC2[:, :], AF.Tanh)
                    hview = nxt[0:64, 1:5, 2:130]
                    nc.vector.tensor_tensor(
                        hview,
                        SO2[:, :].rearrange("p (a b) -> p a b", b=128),
                        TC2[:, :].rearrange("p (a b) -> p a b", b=128),
                        OP.mult)
                    if t % 2 == 1:
                        k = t // 2
                        PA = g2.tile([64, 4, 128], BF, tag="PA2")
                        nc.vector.tensor_tensor(
                            PA[:, :, :], cur[0:64, 1:5, 2:130],
                            nxt[0:64, 1:5, 2:130], OP.max)
                        PAv = PA.rearrange("p a (b c) -> p a b c", c=2)
                        PX = g2.tile([64, 4, 64], BF, tag="PX2")
                        nc.vector.tensor_tensor(
                            PX[:, :, :], PAv[:, :, :, 0], PAv[:, :, :, 1],
                            OP.max)
                        PXv = PX.rearrange("p (a c) b -> p a c b", c=2)
                        nc.vector.tensor_tensor(
                            PL2R[0:64, 2 * k:2 * k + 2, :],
                            PXv[:, :, 0, :], PXv[:, :, 1, :], OP.max)

            nc.vector.tensor_copy(PL2R[64:128, :, 0:63], PL2R[0:64, :, 1:64])

            # ================================================ conv3/4/5 tail
            with (tc.tile_pool(name="psum3", bufs=1, space="PSUM") as ps3,
                  tc.tile_pool(name="tail", bufs=1) as tl):
                Z3 = ps3.tile([14, 256], FP, tag="Z3")
                nmm = 3 * 32
                i = 0
                for kh in range(3):
                    for j in range(32):
                        nc.tensor.matmul(
                            Z3[:, :], PL2R[:, kh:kh + 14, 2 * j],
                            W3[:, kh, 2 * j, :],
                            start=(i == 0), stop=(i == nmm - 1))
                        i += 1
                E0 = tl.tile([14, 256], FP, tag="E0")
                E1 = tl.tile([14, 256], FP, tag="E1")
                E2 = tl.tile([14, 256], FP, tag="E2")
                A3T = tl.tile([14, 256], BF, tag="A3T")
                nc.vector.tensor_tensor(E0[:, :], Z3[:, :], B3R[:, :], OP.add)
                nc.vector.tensor_scalar(E1[:, :], E0[:, :], 0.0, None, OP.min)
                nc.scalar.activation(E1[:, :], E1[:, :], AF.Exp)
                nc.vector.tensor_scalar(E2[:, :], E0[:, :], 0.0, None, OP.max)
                nc.vector.scalar_tensor_tensor(A3T[:, :], E1[:, :], -1.0,
                                               E2[:, :], OP.add, OP.add)
                A3 = tl.tile([128, 2, 14], BF, tag="A3")
                Z3T = ps3.tile([128, 2, 14], BF, tag="Z3T")
                for g in range(2):
                    nc.tensor.transpose(Z3T[:, g, :],
                                        A3T[:, 128 * g:128 * (g + 1)],
                                        IDTB[:, :])
                    nc.scalar.copy(A3[:, g, :], Z3T[:, g, :])
                W4B = tl.tile([128, 2, 128], BF, tag="W4B")
                nc.vector.tensor_copy(W4B.rearrange("p a b -> p (a b)"),
                                      W4.rearrange("p a b -> p (a b)"))
                Z4 = ps3.tile([128, 14], FP, tag="Z4")
                for g in range(2):
                    nc.tensor.matmul(Z4[:, :], W4B[:, g, :], A3[:, g, :],
                                     start=(g == 0), stop=(g == 1))
                F0 = tl.tile([128, 14], FP, tag="F0")
                F1t = tl.tile([128, 14], FP, tag="F1t")
                F2t = tl.tile([128, 14], FP, tag="F2t")
                A4 = tl.tile([128, 14], FP, tag="A4")
                nc.vector.tensor_scalar(F0[:, :], Z4[:, :], B4[:, 0:1], None,
                                        OP.add)
                nc.vector.tensor_scalar(F1t[:, :], F0[:, :], 0.0, None,
                                        OP.min)
                nc.scalar.activation(F1t[:, :], F1t[:, :], AF.Exp)
                nc.vector.tensor_scalar(F2t[:, :], F0[:, :], 0.0, None,
                                        OP.max)
                nc.vector.scalar_tensor_tensor(A4[:, :], F1t[:, :], -1.0,
                                               F2t[:, :], OP.add, OP.add)
                W5B = tl.tile([128, 88], BF, tag="W5B")
                A4B = tl.tile([128, 14], BF, tag="A4B")
                nc.vector.tensor_copy(W5B[:, :], W5[:, :])
                nc.vector.tensor_copy(A4B[:, :], A4[:, :])
                Z5 = ps3.tile([88, 14], FP, tag="Z5")
                nc.tensor.matmul(Z5[:, :], W5B[:, :], A4B[:, :], start=True,
                                 stop=True)
                OUTS = tl.tile([88, 14], FP, tag="OUTS")
                nc.scalar.activation(OUTS[:, :], Z5[:, :], AF.Identity,
                                     bias=B5[:, 0:1])
                dma(out=out_d[:, :], in_=OUTS[:, :])

    _split_waits(nc, mybir)
    return nc


def _split_waits(nc, mybir):
    """neuronxcc codegen allows one embedded sync wait per instruction;
    hoist extra waits into standalone EventSemaphore ops just before."""
    nsplit = 0
    for bb in nc.m.functions[0].blocks:
        new = []
        for inst in bb.instructions:
            si = inst.sync_info
            if si is not None and si.on_wait is not None and len(si.on_wait) > 1:
                waits = list(si.on_wait)
                for w in waits[:-1]:
                    nsplit += 1
                    ev = mybir.InstEventSemaphore(
                        name=f"{inst.name}-sw{nsplit}",
                        engine=inst.engine,
                        sync_info=mybir.SyncInfo(on_wait=[w], on_update=[]),
                    )
                    new.append(ev)
                inst.sync_info = mybir.SyncInfo(
                    on_wait=[waits[-1]], on_update=list(si.on_update or []))
            new.append(inst)
        try:
            bb.instructions = new
        except Exception:
            bb.instructions[:] = new
    return nc


def _prep_weights(w1, b1, w2, b2, w3, b3, w4, b4, w5, b5):
    f = np.float32
    # clstm1: gate rows [i f g o] -> [i f o g]; h-part and x-part split
    perm1 = np.concatenate([np.arange(0, 64), np.arange(96, 128),
                            np.arange(64, 96)])
    w1p = w1[perm1].astype(f).copy()
    b1p = b1[perm1].astype(f).copy()
    wh = np.transpose(w1p[:, 1:33], (1, 2, 3, 0)).reshape(32, 9, 128)
    w1r = np.zeros((128, 6, 128), f)
    w1r[:, 0, :] = np.concatenate([wh[:, 0], wh[:, 1], wh[:, 3], wh[:, 4]])
    w1r[0:64, 1, :] = np.concatenate([wh[:, 6], wh[:, 7]])
    w1r[0:32, 2, :] = wh[:, 2]
    w1r[0:32, 3, :] = wh[:, 5]
    w1r[0:32, 4, :] = wh[:, 8]
    w1r = w1r.reshape(128, 6 * 128)
    w1x = np.transpose(w1p[:, 0], (1, 2, 0)).reshape(9, 128)
    # clstm2: ci rows [h2(0:48), pad(48:64), x(64:96)];
    # co groups A=[f(0:48),-,i(64:112),-], B=[o(0:48),-,g(64:112),-]
    bi, bf_, bg, bo = b2[0:48], b2[48:96], b2[96:144], b2[144:192]
    wi, wf, wg, wo = w2[0:48], w2[48:96], w2[96:144], w2[144:192]
    zpad = np.zeros((16, 80, 3, 3), np.float32)
    wA = np.concatenate([wf, zpad, wi, zpad]).astype(f)     # (128, 80, 3, 3)
    wB = np.concatenate([wo, zpad, wg, zpad]).astype(f)
    wAB = np.concatenate([wA, wB])                          # (256, 80, 3, 3)
    # input-channel remap to [h2, pad, x]
    w2p = np.zeros((256, 96, 3, 3), f)
    w2p[:, 0:48] = wAB[:, 32:80]
    w2p[:, 64:96] = wAB[:, 0:32]
    w2r = np.transpose(w2p, (1, 2, 3, 0)).reshape(96, 9 * 256)
    z16 = np.zeros(16, f)
    b2a = np.concatenate([bf_, z16, bi, z16]).astype(f)
    b2b = np.concatenate([bo, z16, bg, z16]).astype(f)
    # conv3: [128=(ci,parity padded), kh, kw-slot, co]; odd kw at col 2j
    tmp = np.transpose(w3.astype(f), (1, 2, 3, 0))          # (48,3,64,256)
    w3r = np.zeros((128, 3, 64, 256), f)
    w3r[0:48, :, 0::2, :] = tmp[:, :, 0::2, :]
    w3r[64:112, :, 0::2, :] = tmp[:, :, 1::2, :]
    w4r = np.transpose(w4[:, :, 0, 0].astype(f).reshape(128, 2, 128),
                       (2, 1, 0))
    w5r = w5[:, :, 0, 0].astype(f).T
    i32 = np.eye(32, dtype=f)
    ip2 = np.zeros((128, 64), f)
    ip2[0:48, 0:48] = np.eye(48, dtype=f)
    ip2[64:112, 0:48] = np.eye(48, dtype=f)
    cpf = np.zeros((128, 368), f)
    cpf[:, 0] = b1p
    cpf[:, 1] = b2a
    cpf[:, 2] = b2b
    cpf[:, 3] = b4.astype(f)
    cpf[0:88, 4] = b5.astype(f)
    cpf[0:14, 8:22] = np.eye(14, dtype=f)
    cpf[0:14, 22:278] = np.tile(b3.astype(f)[None, :], (14, 1))
    cpf[:, 280:368] = w5r
    cpb = np.zeros((128, 256), f)
    cpb[0:9, 0:128] = w1x
    cpb[0:64, 128:160] = np.vstack([i32, i32])
    cpb[:, 160:224] = ip2
    cpb[0:14, 224:238] = np.eye(14, dtype=f)
    return dict(
        w1r=w1r, w2r=w2r, w3r=w3r.reshape(128, 3 * 64 * 256),
        w4r=np.ascontiguousarray(w4r.reshape(128, 2 * 128)),
        cpf=cpf, cpb=cpb,
    )


def kernel(x, w1, b1, w2, b2, w3, b3, w4, b4, w5, b5):
    import ml_dtypes
    from concourse import bass_utils

    bf16 = ml_dtypes.bfloat16
    if "nc" not in _CACHE:
        _CACHE["nc"] = _build_program()
    nc = _CACHE["nc"]

    wd = _prep_weights(w1, b1, w2, b2, w3, b3, w4, b4, w5, b5)
    shared = {
        "w1r": wd["w1r"].astype(bf16), "w2r": wd["w2r"].astype(bf16),
        "w3r": wd["w3r"].astype(bf16), "w4r": wd["w4r"],
        "cpf": wd["cpf"], "cpb": wd["cpb"].astype(bf16),
    }
    in_maps = []
    for i in range(N_CORES):
        xp = np.zeros((T, PH1, PW1), np.float32)
        xp[:, 1:9, 2:258] = x[i, 0]
        xim = np.zeros((9, T, 8, 256), np.float32)
        for off in range(9):
            dy, dx = off // 3 - 1, off % 3 - 1
            xim[off] = xp[:, 1 + dy:9 + dy, 2 + dx:258 + dx]
        m = dict(shared)
        m["xim"] = xim.reshape(9, T * 2048).astype(bf16)
        in_maps.append(m)

    global _last_in_maps
    _last_in_maps = in_maps
    res = bass_utils.run_bass_kernel_spmd(nc, in_maps,
                                          core_ids=list(range(N_CORES)))
    out = np.stack([r["out"] for r in res.results])
    return out[..., None].astype(np.float32)



# revision 4
# speedup vs baseline: 7.9328x; 7.9328x over previous
"""ConvLSTM net (nn_Net_50354196578736) Trainium2 Bass kernel.

Data-parallel over batch: B=8 -> 1 sample per NeuronCore, 8 cores, no
collectives. Per core:
  clstm1 (T=32, 33->128ch, 3x3 SAME on 8x256) -> maxpool3d 2x2x2
  clstm2 (T=16, 80->192ch, 3x3 SAME on 4x128) -> maxpool3d 2x2x2
  reshape -> conv3 (256,48,3,64) VALID + ELU -> conv4 1x1 + ELU -> conv5 1x1

Conv-as-matmul: channels on partitions, zero-padded spatial planes on the
free dim, fp32 PSUM accumulation over shifted-view matmuls, bf16 datapath.

clstm1 K-stacking: the hidden state h (32ch) is kept in 4 partition
quadrants of the recurrent input buffer - quadrant 0 unshifted plus three
spatially shifted replicas (+1 col, +1 row, +1 row+1 col) built by
background SBUF->SBUF DMAs. Kernel offsets whose spatial deltas match the
replica shifts then stack on the contraction axis, collapsing the 9-offset
3x3 conv to 5 matmul passes: one K=128 (offsets (-1,-1),(-1,0),(0,-1),
(0,0)), one K=64 ((1,-1),(1,0)), three K=32. The x-channel contribution is
host-precomputed as a K=9 im2col and folded in as one more accumulating
matmul. (True tile_position row-tiling was probed and hard-faults when
concurrent row tiles accumulate into one PSUM bank.)

Gate math per step: z rows ordered [i,f,o,g]; one sigmoid scan over
[i,f,o]; tanh(g) straight from PSUM partition-shifted into the [tg; c]
pair tile; one paired tensor_tensor makes [sig(i)*tg; sig(f)*c]; the pair
sum c = m1+m2 runs on the PE via a stacked-identity matmul; tanh(c) lands
partition-shifted next to sig(o) for the h product, which writes the next
step's padded conv input directly.

Partition-alignment rules (verified empirically): ops with a PSUM input
may shift partitions freely; two-SBUF-input tensor_tensor needs equal
input bases (output base free); single-SBUF-input ops shift freely;
TensorCopy/Memset need 32-aligned bases.

_split_waits: this walrus build accepts only one embedded sync wait per
instruction; the pass hoists extra waits into standalone EventSemaphore
ops on the same engine. All DMAs use the single SWDGE queue for the same
reason. Host-side numpy does all weight permutation/padding/packing.
"""

import numpy as np

B, T, H, W = 8, 32, 8, 256
F1, F2, F3, F4, NN = 32, 48, 256, 128, 88
N_CORES = 8

PH1, PW1 = 10, 260   # padded layer1 plane; valid (y,x) at (y+1, x+2)
PH2, PW2 = 6, 132    # padded layer2 plane (4x128 maps)
SP1 = H * W          # 2048
SP2 = 4 * 128        # 512

_CACHE = {}


def _build_program():
    import concourse.bass as bass
    import concourse.mybir as mybir
    from concourse.tile import TileContext

    dt = mybir.dt
    AF = mybir.ActivationFunctionType
    OP = mybir.AluOpType
    BF, FP = dt.bfloat16, dt.float32

    nc = bass.Bass(trn_type="TRN2", target_bir_lowering=True, use_seq_codegen=True)

    xim_d = nc.dram_tensor("xim", [9, T * 2048], BF, kind="ExternalInput")
    w1_d = nc.dram_tensor("w1r", [128, 6 * 128], BF, kind="ExternalInput")
    w2_d = nc.dram_tensor("w2r", [96, 9 * 256], BF, kind="ExternalInput")
    w3_d = nc.dram_tensor("w3r", [128, 3 * 64 * 256], BF, kind="ExternalInput")
    w4_d = nc.dram_tensor("w4r", [128, 2 * 128], FP, kind="ExternalInput")
    cpf_d = nc.dram_tensor("cpf", [128, 368], FP, kind="ExternalInput")
    cpb_d = nc.dram_tensor("cpb", [128, 256], BF, kind="ExternalInput")
    out_d = nc.dram_tensor("out", [88, 14], FP, kind="ExternalOutput")

    with TileContext(nc) as tc:
        with tc.tile_pool(name="persist", bufs=1) as pp:
            W1 = pp.tile([128, 6, 128], BF, tag="W1")
            W2 = pp.tile([96, 9, 256], BF, tag="W2")
            W4 = pp.tile([128, 2, 128], FP, tag="W4")
            CPF = pp.tile([128, 368], FP, tag="CPF")
            CPB = pp.tile([128, 256], BF, tag="CPB")
            B1 = CPF[:, 0:1]
            B2A = CPF[:, 1:2]
            B2B = CPF[:, 2:3]
            B4 = CPF[:, 3:4]
            B5 = CPF[0:88, 4:5]
            IDT = CPF[0:14, 8:22]
            B3R = CPF[0:14, 22:278]
            W5 = CPF[:, 280:368]
            W1X = CPB[0:9, 0:128]
            IP1 = CPB[0:64, 128:160]
            IP2 = CPB[:, 160:224]
            IDTB = CPB[0:14, 224:238]
            INb = [pp.tile([128, PH1, PW1], BF, tag=f"IN{k}", name=f"IN{k}")
                   for k in range(2)]
            IN2b = [pp.tile([96, PH2, PW2], BF, tag=f"IN2{k}", name=f"IN2{k}")
                    for k in range(2)]
            TGC1 = pp.tile([64, SP1], BF, tag="TGC1")    # [tg ; c]
            TGC2 = pp.tile([128, SP2], BF, tag="TGC2")   # [c2,-,tg2,-]
            XP2 = pp.tile([32, 16, 512], BF, tag="XP2")
            PL2R = pp.tile([128, 16, 64], BF, tag="PL2R")

            dma = nc.gpsimd.dma_start
            dma(out=W1.rearrange("p a b -> p (a b)"), in_=w1_d[:, :])
            dma(out=W2.rearrange("p a b -> p (a b)"), in_=w2_d[:, :])
            dma(out=W4.rearrange("p a b -> p (a b)"), in_=w4_d[:, :])
            dma(out=CPF[:, :], in_=cpf_d[:, :])
            dma(out=CPB[:, :], in_=cpb_d[:, :])

            for k in range(2):
                nc.vector.memset(INb[k].rearrange("p a b -> p (a b)"), 0.0)
                nc.vector.memset(IN2b[k].rearrange("p a b -> p (a b)"), 0.0)
            nc.vector.memset(TGC1[:, :], 0.0)
            nc.vector.memset(TGC2[:, :], 0.0)

            # ============================= clstm1, 32 steps x 2 half-planes
            with (tc.tile_pool(name="psum1", bufs=2, space="PSUM") as ps1,
                  tc.tile_pool(name="ximp", bufs=2) as xp,
                  tc.tile_pool(name="gates1", bufs=3) as g1):
                S = g1.tile([128, SP1], BF, tag="S1", bufs=1)
                TC = g1.tile([96, SP1], BF, tag="TC", bufs=1)
                # preheat: absorb init-DMA sem into each engine's clock so
                # steady-state instructions carry <=2 sync waits
                PHP = ps1.tile([2, 4], FP, tag="Z1")
                nc.tensor.matmul(PHP[:, :], CPB[0:9, 0:2], CPB[0:9, 0:4],
                                 start=True, stop=True)
                nc.scalar.copy(S[0:2, 0:2], CPF[0:2, 0:2])
                nc.vector.tensor_copy(TGC1[0:2, 0:2], CPF[0:2, 0:2])
                for t in range(T):
                    if t % 2 == 0:
                        XIMc = xp.tile([9, 2, 2, 1024], BF, tag="XIMc",
                                       name="XIMc")
                        dma(out=XIMc.rearrange("p a b c -> p (a b c)"),
                            in_=xim_d[:, 2048 * t:2048 * (t + 2)])
                    cur, nxt = INb[t % 2], INb[(t + 1) % 2]
                    for hf in range(2):
                        hs = slice(1024 * hf, 1024 * (hf + 1))
                        Z = ps1.tile([128, 4, 256], FP, tag="Z1")
                        Zq = Z.rearrange("p a b -> p (a b)")
                        for q in range(2):
                            nc.tensor.matmul(
                                Zq[:, 512 * q:512 * (q + 1)],
                                W1X[:, :],
                                XIMc[:, t % 2, hf, 512 * q:512 * (q + 1)],
                                start=True, stop=False)
                        groups = ((0, 128, -1, -1), (1, 64, 1, -1),
                                  (2, 32, -1, 1), (3, 32, 0, 1),
                                  (4, 32, 1, 1))
                        for y in range(4):
                            yy = 4 * hf + y
                            for gi, (slot, K, dy, dx) in enumerate(groups):
                                nc.tensor.matmul(
                                    Z[:, y, :],
                                    W1[0:K, slot, :],
                                    cur[0:K, yy + 1 + dy, 2 + dx:2 + dx + 256],
                                    start=False, stop=(gi == 4))
                        Zf = Z.rearrange("p a b -> p (a b)")
                        nc.scalar.activation(S[0:96, hs], Zf[0:96, :], AF.Sigmoid,
                                             bias=B1[0:96, 0:1])
                        nc.scalar.activation(TGC1[0:32, hs], Zf[96:128, :],
                                             AF.Tanh, bias=B1[96:128, 0:1])
                        P2 = g1.tile([64, 1024], BF, tag="P2")
                        nc.vector.tensor_tensor(P2[:, :], S[0:64, hs],
                                                TGC1[:, hs], OP.mult)
                        ZC = ps1.tile([32, 1024], FP, tag="ZC")
                        for q in range(2):
                            nc.tensor.matmul(ZC[:, 512 * q:512 * (q + 1)],
                                             IP1[:, :],
                                             P2[:, 512 * q:512 * (q + 1)],
                                             start=True, stop=True)
                        nc.vector.tensor_copy(TGC1[32:64, hs], ZC[:, :])
                        nc.scalar.activation(TC[64:96, hs], ZC[:, :], AF.Tanh)
                        hview = nxt[0:32, 1 + 4 * hf:5 + 4 * hf, 2:258]
                        nc.vector.tensor_tensor(
                            hview,
                            S[64:96, hs].rearrange("p (a b) -> p a b", b=256),
                            TC[64:96, hs].rearrange("p (a b) -> p a b", b=256),
                            OP.mult)
                        r0, r1 = 1 + 4 * hf, 5 + 4 * hf
                        dma(out=nxt[32:64, r0:r1, 1:257], in_=hview)
                        dma(out=nxt[64:96, r0 - 1:r1 - 1, 2:258], in_=hview)
                        dma(out=nxt[96:128, r0 - 1:r1 - 1, 1:257], in_=hview)
                    if t % 2 == 1:
                        k = t // 2
                        PA = g1.tile([32, 8, 256], BF, tag="PA")
                        nc.vector.tensor_tensor(
                            PA[:, :, :], cur[0:32, 1:9, 2:258],
                            nxt[0:32, 1:9, 2:258], OP.max)
                        PAv = PA.rearrange("p a (b c) -> p a b c", c=2)
                        PX = g1.tile([32, 8, 128], BF, tag="PX")
                        nc.vector.tensor_tensor(
                            PX[:, :, :], PAv[:, :, :, 0], PAv[:, :, :, 1],
                            OP.max)
                        PXv = PX.rearrange("p (a c) b -> p a c b", c=2)
                        XPv = XP2.rearrange("p a (h w) -> p a h w", w=128)
                        nc.vector.tensor_tensor(
                            XPv[:, k, :, :],
                            PXv[:, :, 0, :], PXv[:, :, 1, :], OP.max)

            # ================================================ clstm2, 16 steps
            W3 = pp.tile([128, 3, 64, 256], BF, tag="W3")
            dma(out=W3.rearrange("p a b c -> p (a b c)"), in_=w3_d[:, :])
            with (tc.tile_pool(name="psum2", bufs=2, space="PSUM") as ps2,
                  tc.tile_pool(name="gates2", bufs=3) as g2):
                for t in range(16):
                    cur, nxt = IN2b[t % 2], IN2b[(t + 1) % 2]
                    nc.vector.tensor_copy(
                        cur[64:96, 1:5, 2:130],
                        XP2[:, t, :].rearrange("p (a b) -> p a b", b=128))
                    ZA = ps2.tile([128, SP2], FP, tag="ZA")
                    ZB = ps2.tile([128, SP2], FP, tag="ZB")
                    for zt, c0 in ((ZA, 0), (ZB, 128)):
                        for off in range(9):
                            dy, dx = off // 3 - 1, off % 3 - 1
                            rhs = cur[:, 1 + dy:5 + dy, 2 + dx:2 + dx + 128]
                            nc.tensor.matmul(zt[:, :], W2[:, off, c0:c0 + 128],
                                             rhs, start=(off == 0),
                                             stop=(off == 8))
                    # ZA rows [f(0:48) - i(64:112) -]; ZB [o(0:48) - g(64:112) -]
                    S2 = g2.tile([128, SP2], BF, tag="S2")
                    SO2 = g2.tile([64, SP2], BF, tag="SO2")
                    nc.scalar.activation(S2[:, :], ZA[:, :], AF.Sigmoid,
                                         bias=B2A[:, 0:1])
                    nc.scalar.activation(SO2[:, :], ZB[0:64, :], AF.Sigmoid,
                                         bias=B2B[0:64, 0:1])
                    nc.scalar.activation(TGC2[64:128, :], ZB[64:128, :],
                                         AF.Tanh, bias=B2B[64:128, 0:1])
                    P22 = g2.tile([128, SP2], BF, tag="P22")
                    nc.vector.tensor_tensor(P22[:, :], S2[:, :], TGC2[:, :],
                                            OP.mult)
                    ZC2 = ps2.tile([64, SP2], FP, tag="ZC2")
                    nc.tensor.matmul(ZC2[:, :], IP2[:, :], P22[:, :],
                                     start=True, stop=True)
                    nc.vector.tensor_copy(TGC2[0:64, :], ZC2[:, :])
                    TC2 = g2.tile([64, SP2], BF, tag="TC2")
                    nc.scalar.activation(TC2[:, :], ZC2[:, :], AF.Tanh)
                    hview = nxt[0:64, 1:5, 2:130]
                    nc.vector.tensor_tensor(
                        hview,
                        SO2[:, :].rearrange("p (a b) -> p a b", b=128),
                        TC2[:, :].rearrange("p (a b) -> p a b", b=128),
                        OP.mult)
                    if t % 2 == 1:
                        k = t // 2
                        PA = g2.tile([64, 4, 128], BF, tag="PA2")
                        nc.vector.tensor_tensor(
                            PA[:, :, :], cur[0:64, 1:5, 2:130],
                            nxt[0:64, 1:5, 2:130], OP.max)
                        PAv = PA.rearrange("p a (b c) -> p a b c", c=2)
                        PX = g2.tile([64, 4, 64], BF, tag="PX2")
                        nc.vector.tensor_tensor(
                            PX[:, :, :], PAv[:, :, :, 0], PAv[:, :, :, 1],
                            OP.max)
                        PXv = PX.rearrange("p (a c) b -> p a c b", c=2)
                        nc.vector.tensor_tensor(
                            PL2R[0:64, 2 * k:2 * k + 2, :],
                            PXv[:, :, 0, :], PXv[:, :, 1, :], OP.max)

            nc.vector.tensor_copy(PL2R[64:128, :, 0:63], PL2R[0:64, :, 1:64])

            # ================================================ conv3/4/5 tail
            with (tc.tile_pool(name="psum3", bufs=1, space="PSUM") as ps3,
                  tc.tile_pool(name="tail", bufs=1) as tl):
                Z3 = ps3.tile([14, 256], FP, tag="Z3")
                nmm = 3 * 32
                i = 0
                for kh in range(3):
                    for j in range(32):
                        nc.tensor.matmul(
                            Z3[:, :], PL2R[:, kh:kh + 14, 2 * j],
                            W3[:, kh, 2 * j, :],
                            start=(i == 0), stop=(i == nmm - 1))
                        i += 1
                E0 = tl.tile([14, 256], FP, tag="E0")
                E1 = tl.tile([14, 256], FP, tag="E1")
                E2 = tl.tile([14, 256], FP, tag="E2")
                A3T = tl.tile([14, 256], BF, tag="A3T")
                nc.vector.tensor_tensor(E0[:, :], Z3[:, :], B3R[:, :], OP.add)
                nc.vector.tensor_scalar(E1[:, :], E0[:, :], 0.0, None, OP.min)
                nc.scalar.activation(E1[:, :], E1[:, :], AF.Exp)
                nc.vector.tensor_scalar(E2[:, :], E0[:, :], 0.0, None, OP.max)
                nc.vector.scalar_tensor_tensor(A3T[:, :], E1[:, :], -1.0,
                                               E2[:, :], OP.add, OP.add)
                A3 = tl.tile([128, 2, 14], BF, tag="A3")
                Z3T = ps3.tile([128, 2, 14], BF, tag="Z3T")
                for g in range(2):
                    nc.tensor.transpose(Z3T[:, g, :],
                                        A3T[:, 128 * g:128 * (g + 1)],
                                        IDTB[:, :])
                    nc.scalar.copy(A3[:, g, :], Z3T[:, g, :])
                W4B = tl.tile([128, 2, 128], BF, tag="W4B")
                nc.vector.tensor_copy(W4B.rearrange("p a b -> p (a b)"),
                                      W4.rearrange("p a b -> p (a b)"))
                Z4 = ps3.tile([128, 14], FP, tag="Z4")
                for g in range(2):
                    nc.tensor.matmul(Z4[:, :], W4B[:, g, :], A3[:, g, :],
                                     start=(g == 0), stop=(g == 1))
                F0 = tl.tile([128, 14], FP, tag="F0")
                F1t = tl.tile([128, 14], FP, tag="F1t")
                F2t = tl.tile([128, 14], FP, tag="F2t")
                A4 = tl.tile([128, 14], FP, tag="A4")
                nc.vector.tensor_scalar(F0[:, :], Z4[:, :], B4[:, 0:1], None,
                                        OP.add)
                nc.vector.tensor_scalar(F1t[:, :], F0[:, :], 0.0, None,
                                        OP.min)
                nc.scalar.activation(F1t[:, :], F1t[:, :], AF.Exp)
                nc.vector.tensor_scalar(F2t[:, :], F0[:, :], 0.0, None,
                                        OP.max)
                nc.vector.scalar_tensor_tensor(A4[:, :], F1t[:, :], -1.0,
                                               F2t[:, :], OP.add, OP.add)
                W5B = tl.tile([128, 88], BF, tag="W5B")
                A4B = tl.tile([128, 14], BF, tag="A4B")
                nc.vector.tensor_copy(W5B[:, :], W5[:, :])
                nc.vector.tensor_copy(A4B[:, :], A4[:, :])
                Z5 = ps3.tile([88, 14], FP, tag="Z5")
                nc.tensor.matmul(Z5[:, :], W5B[:, :], A4B[:, :], start=True,
                                 stop=True)
                OUTS = tl.tile([88, 14], FP, tag="OUTS")
                nc.scalar.activation(OUTS[:, :], Z5[:, :], AF.Identity,
                                     bias=B5[:, 0:1])
                dma(out=out_d[:, :], in_=OUTS[:, :])

    _split_waits(nc, mybir)
    return nc


def _split_waits(nc, mybir):
    """neuronxcc codegen allows one embedded sync wait per instruction;
    hoist extra waits into standalone EventSemaphore ops just before."""
    nsplit = 0
    for bb in nc.m.functions[0].blocks:
        new = []
        for inst in bb.instructions:
            si = inst.sync_info
            if si is not None and si.on_wait is not None and len(si.on_wait) > 1:
                waits = list(si.on_wait)
                for w in waits[:-1]:
                    nsplit += 1
                    ev = mybir.InstEventSemaphore(
                        name=f"{inst.name}-sw{nsplit}",
                        engine=inst.engine,
                        sync_info=mybir.SyncInfo(on_wait=[w], on_update=[]),
                    )
                    new.append(ev)
                inst.sync_info = mybir.SyncInfo(
                    on_wait=[waits[-1]], on_update=list(si.on_update or []))
            new.append(inst)
        try:
            bb.instructions = new
        except Exception:
            bb.instructions[:] = new
    return nc


def _prep_weights(w1, b1, w2, b2, w3, b3, w4, b4, w5, b5):
    f = np.float32
    # clstm1: gate rows [i f g o] -> [i f o g]; h-part and x-part split
    perm1 = np.concatenate([np.arange(0, 64), np.arange(96, 128),
                            np.arange(64, 96)])
    w1p = w1[perm1].astype(f).copy()
    b1p = b1[perm1].astype(f).copy()
    wh = np.transpose(w1p[:, 1:33], (1, 2, 3, 0)).reshape(32, 9, 128)
    w1r = np.zeros((128, 6, 128), f)
    w1r[:, 0, :] = np.concatenate([wh[:, 0], wh[:, 1], wh[:, 3], wh[:, 4]])
    w1r[0:64, 1, :] = np.concatenate([wh[:, 6], wh[:, 7]])
    w1r[0:32, 2, :] = wh[:, 2]
    w1r[0:32, 3, :] = wh[:, 5]
    w1r[0:32, 4, :] = wh[:, 8]
    w1r = w1r.reshape(128, 6 * 128)
    w1x = np.transpose(w1p[:, 0], (1, 2, 0)).reshape(9, 128)
    # clstm2: ci rows [h2(0:48), pad(48:64), x(64:96)];
    # co groups A=[f(0:48),-,i(64:112),-], B=[o(0:48),-,g(64:112),-]
    bi, bf_, bg, bo = b2[0:48], b2[48:96], b2[96:144], b2[144:192]
    wi, wf, wg, wo = w2[0:48], w2[48:96], w2[96:144], w2[144:192]
    zpad = np.zeros((16, 80, 3, 3), np.float32)
    wA = np.concatenate([wf, zpad, wi, zpad]).astype(f)     # (128, 80, 3, 3)
    wB = np.concatenate([wo, zpad, wg, zpad]).astype(f)
    wAB = np.concatenate([wA, wB])                          # (256, 80, 3, 3)
    # input-channel remap to [h2, pad, x]
    w2p = np.zeros((256, 96, 3, 3), f)
    w2p[:, 0:48] = wAB[:, 32:80]
    w2p[:, 64:96] = wAB[:, 0:32]
    w2r = np.transpose(w2p, (1, 2, 3, 0)).reshape(96, 9 * 256)
    z16 = np.zeros(16, f)
    b2a = np.concatenate([bf_, z16, bi, z16]).astype(f)
    b2b = np.concatenate([bo, z16, bg, z16]).astype(f)
    # conv3: [128=(ci,parity padded), kh, kw-slot, co]; odd kw at col 2j
    tmp = np.transpose(w3.astype(f), (1, 2, 3, 0))          # (48,3,64,256)
    w3r = np.zeros((128, 3, 64, 256), f)
    w3r[0:48, :, 0::2, :] = tmp[:, :, 0::2, :]
    w3r[64:112, :, 0::2, :] = tmp[:, :, 1::2, :]
    w4r = np.transpose(w4[:, :, 0, 0].astype(f).reshape(128, 2, 128),
                       (2, 1, 0))
    w5r = w5[:, :, 0, 0].astype(f).T
    i32 = np.eye(32, dtype=f)
    ip2 = np.zeros((128, 64), f)
    ip2[0:48, 0:48] = np.eye(48, dtype=f)
    ip2[64:112, 0:48] = np.eye(48, dtype=f)
    cpf = np.zeros((128, 368), f)
    cpf[:, 0] = b1p
    cpf[:, 1] = b2a
    cpf[:, 2] = b2b
    cpf[:, 3] = b4.astype(f)
    cpf[0:88, 4] = b5.astype(f)
    cpf[0:14, 8:22] = np.eye(14, dtype=f)
    cpf[0:14, 22:278] = np.tile(b3.astype(f)[None, :], (14, 1))
    cpf[:, 280:368] = w5r
    cpb = np.zeros((128, 256), f)
    cpb[0:9, 0:128] = w1x
    cpb[0:64, 128:160] = np.vstack([i32, i32])
    cpb[:, 160:224] = ip2
    cpb[0:14, 224:238] = np.eye(14, dtype=f)
    return dict(
        w1r=w1r, w2r=w2r, w3r=w3r.reshape(128, 3 * 64 * 256),
        w4r=np.ascontiguousarray(w4r.reshape(128, 2 * 128)),
        cpf=cpf, cpb=cpb,
    )


def _get_runner():
    """Build (once) a cached jitted SPMD dispatcher around _bass_exec_p.

    bass_utils.run_bass_kernel_spmd constructs a fresh closure + jax.jit
    object every call, so each dispatch pays full retrace / XLA compile /
    executable load (~2 s). Building the shard_map'd jit once and caching
    it drops steady-state dispatch to transfer + execute."""
    if "runner" in _CACHE:
        return _CACHE["runner"]
    import jax
    from jax.sharding import Mesh, PartitionSpec, NamedSharding
    from jax.experimental.shard_map import shard_map
    import concourse.mybir as mybir
    from concourse import bass2jax

    if "nc" not in _CACHE:
        _CACHE["nc"] = _build_program()
    nc = _CACHE["nc"]
    assert nc.dbg_addr is None
    part_name = (nc.partition_id_tensor.name
                 if nc.partition_id_tensor is not None else None)

    bass2jax.install_neuronx_cc_hook()

    in_names, out_names, out_avals, zero_shapes = [], [], [], []
    for alloc in nc.m.functions[0].allocations:
        if not isinstance(alloc, mybir.MemoryLocationSet):
            continue
        name = alloc.memorylocations[0].name
        if alloc.kind == "ExternalInput":
            if name != part_name:
                in_names.append(name)
        elif alloc.kind == "ExternalOutput":
            shape = tuple(alloc.tensor_shape)
            dtype = mybir.dt.np(alloc.dtype)
            out_names.append(name)
            out_avals.append(jax.core.ShapedArray(shape, dtype))
            zero_shapes.append((shape, dtype))

    n_params = len(in_names)
    n_outs = len(out_names)
    all_in_names = in_names + out_names
    if part_name is not None:
        all_in_names = all_in_names + [part_name]

    def _body(*args):
        operands = list(args)
        if part_name is not None:
            operands.append(bass2jax.partition_id_tensor())
        outs = bass2jax._bass_exec_p.bind(
            *operands,
            out_avals=tuple(out_avals),
            in_names=tuple(all_in_names),
            out_names=tuple(out_names),
            lowering_input_output_aliases=(),
            sim_require_finite=True,
            sim_require_nnan=True,
            nc=nc,
        )
        return tuple(outs)

    devices = jax.devices()[:N_CORES]
    mesh = Mesh(np.asarray(devices), ("core",))
    spec = PartitionSpec("core")
    fn = jax.jit(
        shard_map(_body, mesh=mesh,
                  in_specs=(spec,) * (n_params + n_outs),
                  out_specs=(spec,) * n_outs, check_rep=False),
        donate_argnums=tuple(range(n_params, n_params + n_outs)),
        keep_unused=True)
    runner = dict(fn=fn, in_names=in_names, zero_shapes=zero_shapes,
                  sharding=NamedSharding(mesh, spec), jax=jax)
    _CACHE["runner"] = runner
    return runner


def kernel(x, w1, b1, w2, b2, w3, b3, w4, b4, w5, b5):
    import hashlib
    import ml_dtypes

    bf16 = ml_dtypes.bfloat16
    r = _get_runner()
    jx = r["jax"]

    # Device-cache the (replicated) weights keyed on content; steady-state
    # calls only re-ship the activations.
    h = hashlib.blake2b(digest_size=16)
    for a in (w1, b1, w2, b2, w3, b3, w4, b4, w5, b5):
        h.update(np.ascontiguousarray(a).tobytes())
    wkey = h.hexdigest()
    if _CACHE.get("wkey") != wkey:
        wd = _prep_weights(w1, b1, w2, b2, w3, b3, w4, b4, w5, b5)
        shared = {
            "w1r": wd["w1r"].astype(bf16), "w2r": wd["w2r"].astype(bf16),
            "w3r": wd["w3r"].astype(bf16), "w4r": wd["w4r"],
            "cpf": wd["cpf"], "cpb": wd["cpb"].astype(bf16),
        }
        wdev = {}
        for name, arr in shared.items():
            ga = np.broadcast_to(arr[None], (N_CORES,) + arr.shape)
            ga = np.ascontiguousarray(ga).reshape(N_CORES * arr.shape[0],
                                                  arr.shape[1])
            wdev[name] = jx.device_put(ga, r["sharding"])
        _CACHE["wdev"] = wdev
        _CACHE["wkey"] = wkey
    wdev = _CACHE["wdev"]

    # per-core x im2col (9 shifted copies of the zero-padded plane)
    xb = x[:, 0].astype(bf16)                      # (8, T, 8, 256)
    xp = np.zeros((N_CORES, T, PH1, PW1), bf16)
    xp[:, :, 1:9, 2:258] = xb
    xim = np.empty((N_CORES, 9, T, 8, 256), bf16)
    for off in range(9):
        dy, dx = off // 3 - 1, off % 3 - 1
        xim[:, off] = xp[:, :, 1 + dy:9 + dy, 2 + dx:258 + dx]
    xim_g = xim.reshape(N_CORES * 9, T * 2048)

    args = [xim_g if name == "xim" else wdev[name] for name in r["in_names"]]
    zeros = [np.zeros((N_CORES * s[0],) + s[1:], dt)
             for (s, dt) in r["zero_shapes"]]
    outs = r["fn"](*args, *zeros)
    out = np.asarray(outs[0]).reshape(N_CORES, 88, 14)
    return out[..., None].astype(np.float32)



# revision 18
# speedup vs baseline: 31.0584x; 3.9152x over previous
"""ConvLSTM net (nn_Net_50354196578736) Trainium2 Bass kernel.

Data-parallel over batch: B=8 -> 1 sample per NeuronCore, 8 cores, no
collectives. Per core:
  clstm1 (T=32, 33->128ch, 3x3 SAME on 8x256) -> maxpool3d 2x2x2
  clstm2 (T=16, 80->192ch, 3x3 SAME on 4x128) -> maxpool3d 2x2x2
  reshape -> conv3 (256,48,3,64) VALID + ELU -> conv4 1x1 + ELU -> conv5 1x1

Conv-as-matmul: channels on partitions, zero-padded spatial planes on the
free dim, fp32 PSUM accumulation over shifted-view matmuls, bf16 datapath.

clstm1 K-stacking: the hidden state h (32ch) is kept in 4 partition
quadrants of the recurrent input buffer - quadrant 0 unshifted plus three
spatially shifted replicas (+1 col, +1 row, +1 row+1 col) built by
background SBUF->SBUF DMAs. Kernel offsets whose spatial deltas match the
replica shifts then stack on the contraction axis, collapsing the 9-offset
3x3 conv to 5 matmul passes: one K=128 (offsets (-1,-1),(-1,0),(0,-1),
(0,0)), one K=64 ((1,-1),(1,0)), three K=32. The x-channel contribution is
host-precomputed as a K=9 im2col and folded in as one more accumulating
matmul. (True tile_position row-tiling was probed and hard-faults when
concurrent row tiles accumulate into one PSUM bank.)

Gate math per step: z rows ordered [i,f,o,g]; one sigmoid scan over
[i,f,o]; tanh(g) straight from PSUM partition-shifted into the [tg; c]
pair tile; one paired tensor_tensor makes [sig(i)*tg; sig(f)*c]; the pair
sum c = m1+m2 runs on the PE via a stacked-identity matmul; tanh(c) lands
partition-shifted next to sig(o) for the h product, which writes the next
step's padded conv input directly.

Partition-alignment rules (verified empirically): ops with a PSUM input
may shift partitions freely; two-SBUF-input tensor_tensor needs equal
input bases (output base free); single-SBUF-input ops shift freely;
TensorCopy/Memset need 32-aligned bases.

_split_waits: this walrus build accepts only one embedded sync wait per
instruction; the pass hoists extra waits into standalone EventSemaphore
ops on the same engine. All DMAs use the single SWDGE queue for the same
reason. Host-side numpy does all weight permutation/padding/packing.
"""

import numpy as np

B, T, H, W = 8, 32, 8, 256
F1, F2, F3, F4, NN = 32, 48, 256, 128, 88
N_CORES = 8

PH1, PW1 = 10, 260   # padded layer1 plane; valid (y,x) at (y+1, x+2)
PH2, PW2 = 6, 132    # padded layer2 plane (4x128 maps)
SP1 = H * W          # 2048
SP2 = 4 * 128        # 512

_CACHE = {}


def _build_program():
    import concourse.bass as bass
    import concourse.mybir as mybir
    from concourse.tile import TileContext

    dt = mybir.dt
    AF = mybir.ActivationFunctionType
    OP = mybir.AluOpType
    BF, FP = dt.bfloat16, dt.float32

    nc = bass.Bass(trn_type="TRN2", target_bir_lowering=True, use_seq_codegen=True)

    xpd_d = nc.dram_tensor("xpad", [T, 2600], BF, kind="ExternalInput")
    w1_d = nc.dram_tensor("w1r", [128, 6 * 128], BF, kind="ExternalInput")
    w2_d = nc.dram_tensor("w2r", [96, 9 * 256], BF, kind="ExternalInput")
    w3_d = nc.dram_tensor("w3r", [128, 3 * 32 * 256], BF, kind="ExternalInput")
    w4_d = nc.dram_tensor("w4r", [128, 2 * 128], FP, kind="ExternalInput")
    cpf_d = nc.dram_tensor("cpf", [128, 368], FP, kind="ExternalInput")
    cpb_d = nc.dram_tensor("cpb", [128, 256], BF, kind="ExternalInput")
    out_d = nc.dram_tensor("out", [88, 14], FP, kind="ExternalOutput")

    with TileContext(nc) as tc:
        with tc.tile_pool(name="persist", bufs=1) as pp:
            W1 = pp.tile([128, 6, 128], BF, tag="W1")
            W2 = pp.tile([96, 9, 256], BF, tag="W2")
            W4 = pp.tile([128, 2, 128], FP, tag="W4")
            CPF = pp.tile([128, 368], FP, tag="CPF")
            CPB = pp.tile([128, 256], BF, tag="CPB")
            B1 = CPF[:, 0:1]
            B2A = CPF[:, 1:2]
            B2B = CPF[:, 2:3]
            B4 = CPF[:, 3:4]
            B5 = CPF[0:88, 4:5]
            IDT = CPF[0:14, 8:22]
            B3R = CPF[0:14, 22:278]
            W5 = CPF[:, 280:368]
            W1X = CPB[0:9, 0:128]
            IP1 = CPB[0:64, 128:160]
            IP2 = CPB[:, 160:224]
            IDTB = CPB[0:14, 224:238]
            INb = [pp.tile([128, PH1, PW1], BF, tag=f"IN{k}", name=f"IN{k}")
                   for k in range(2)]
            IN2b = [pp.tile([96, PH2, PW2], BF, tag=f"IN2{k}", name=f"IN2{k}")
                    for k in range(2)]
            TGC1 = pp.tile([64, SP1], BF, tag="TGC1")    # [tg ; c]
            TGC2 = pp.tile([128, SP2], BF, tag="TGC2")   # [c2,-,tg2,-]
            XP2 = pp.tile([32, 16, 512], BF, tag="XP2")
            PL2R = pp.tile([128, 16, 64], BF, tag="PL2R")

            dma = nc.gpsimd.dma_start
            dma(out=W1.rearrange("p a b -> p (a b)"), in_=w1_d[:, :])
            dma(out=W2.rearrange("p a b -> p (a b)"), in_=w2_d[:, :])
            dma(out=W4.rearrange("p a b -> p (a b)"), in_=w4_d[:, :])
            dma(out=CPF[:, :], in_=cpf_d[:, :])
            dma(out=CPB[:, :], in_=cpb_d[:, :])

            for k in range(2):
                nc.vector.memset(INb[k].rearrange("p a b -> p (a b)"), 0.0)
                nc.vector.memset(IN2b[k].rearrange("p a b -> p (a b)"), 0.0)
            nc.vector.memset(TGC1[:, :], 0.0)
            nc.vector.memset(TGC2[:, :], 0.0)

            # ============================= clstm1, 32 steps x 2 half-planes
            with (tc.tile_pool(name="psum1", bufs=2, space="PSUM") as ps1,
                  tc.tile_pool(name="ximp", bufs=1) as xp,
                  tc.tile_pool(name="gates1", bufs=3) as g1):
                S = g1.tile([128, SP1], BF, tag="S1", bufs=1)
                TC = g1.tile([96, SP1], BF, tag="TC", bufs=1)
                # on-device im2col of x: 9 shifted windows of the padded
                # (10x260) planes land on partitions 0-8 of XIM
                XIM = xp.tile([9, T * 2048], BF, tag="XIM")
                XIMv = XIM.rearrange("p (t h w) -> p t h w", t=T, h=8, w=256)
                vx = xpd_d.rearrange("t (h w) -> t h w", h=10, w=260)
                for off in range(9):
                    dy, dx = off // 3 - 1, off % 3 - 1
                    dma(out=XIMv[off:off + 1, :, :, :],
                        in_=vx[:, 1 + dy:9 + dy,
                               2 + dx:258 + dx].unsqueeze(0))
                XIMq = XIM.rearrange("p (t a) -> p t a", t=T)
                # preheat: absorb init-DMA sem into each engine's clock so
                # steady-state instructions carry <=2 sync waits
                PHP = ps1.tile([2, 4], FP, tag="Z1")
                nc.tensor.matmul(PHP[:, :], CPB[0:9, 0:2], CPB[0:9, 0:4],
                                 start=True, stop=True)
                nc.scalar.copy(S[0:2, 0:2], CPF[0:2, 0:2])
                nc.vector.tensor_copy(TGC1[0:2, 0:2], CPF[0:2, 0:2])
                for t in range(T):
                    cur, nxt = INb[t % 2], INb[(t + 1) % 2]
                    for hf in range(2):
                        hs = slice(1024 * hf, 1024 * (hf + 1))
                        Z = ps1.tile([128, 4, 256], FP, tag="Z1")
                        Zq = Z.rearrange("p a b -> p (a b)")
                        for q in range(2):
                            nc.tensor.matmul(
                                Zq[:, 512 * q:512 * (q + 1)],
                                W1X[:, :],
                                XIMq[:, t, 1024 * hf + 512 * q:
                                     1024 * hf + 512 * (q + 1)],
                                start=True, stop=False)
                        groups = ((0, 128, -1, -1), (1, 64, 1, -1),
                                  (2, 32, -1, 1), (3, 32, 0, 1),
                                  (4, 32, 1, 1))
                        for y in range(4):
                            yy = 4 * hf + y
                            for gi, (slot, K, dy, dx) in enumerate(groups):
                                nc.tensor.matmul(
                                    Z[:, y, :],
                                    W1[0:K, slot, :],
                                    cur[0:K, yy + 1 + dy, 2 + dx:2 + dx + 256],
                                    start=False, stop=(gi == 4))
                        Zf = Z.rearrange("p a b -> p (a b)")
                        nc.scalar.activation(S[0:96, hs], Zf[0:96, :], AF.Sigmoid,
                                             bias=B1[0:96, 0:1])
                        nc.scalar.activation(TGC1[0:32, hs], Zf[96:128, :],
                                             AF.Tanh, bias=B1[96:128, 0:1])
                        P2 = g1.tile([64, 1024], BF, tag="P2", bufs=2)
                        nc.vector.tensor_tensor(P2[:, :], S[0:64, hs],
                                                TGC1[:, hs], OP.mult)
                        ZC = ps1.tile([32, 1024], FP, tag="ZC")
                        for q in range(2):
                            nc.tensor.matmul(ZC[:, 512 * q:512 * (q + 1)],
                                             IP1[:, :],
                                             P2[:, 512 * q:512 * (q + 1)],
                                             start=True, stop=True)
                        nc.vector.tensor_copy(TGC1[32:64, hs], ZC[:, :])
                        nc.scalar.activation(TC[64:96, hs], ZC[:, :], AF.Tanh)
                        hview = nxt[0:32, 1 + 4 * hf:5 + 4 * hf, 2:258]
                        nc.vector.tensor_tensor(
                            hview,
                            S[64:96, hs].rearrange("p (a b) -> p a b", b=256),
                            TC[64:96, hs].rearrange("p (a b) -> p a b", b=256),
                            OP.mult)
                        r0, r1 = 1 + 4 * hf, 5 + 4 * hf
                        dma(out=nxt[32:64, r0:r1, 1:257], in_=hview)
                        dma(out=nxt[64:96, r0 - 1:r1 - 1, 2:258], in_=hview)
                        dma(out=nxt[96:128, r0 - 1:r1 - 1, 1:257], in_=hview)
                    if t % 2 == 1:
                        k = t // 2
                        PA = g1.tile([32, 8, 256], BF, tag="PA", bufs=2)
                        nc.vector.tensor_tensor(
                            PA[:, :, :], cur[0:32, 1:9, 2:258],
                            nxt[0:32, 1:9, 2:258], OP.max)
                        PAv = PA.rearrange("p a (b c) -> p a b c", c=2)
                        PX = g1.tile([32, 8, 128], BF, tag="PX", bufs=2)
                        nc.vector.tensor_tensor(
                            PX[:, :, :], PAv[:, :, :, 0], PAv[:, :, :, 1],
                            OP.max)
                        PXv = PX.rearrange("p (a c) b -> p a c b", c=2)
                        XPv = XP2.rearrange("p a (h w) -> p a h w", w=128)
                        nc.vector.tensor_tensor(
                            XPv[:, k, :, :],
                            PXv[:, :, 0, :], PXv[:, :, 1, :], OP.max)

            # ================================================ clstm2, 16 steps
            lp_cm = tc.tile_pool(name="late", bufs=1)
            lp = lp_cm.__enter__()
            W3 = lp.tile([128, 3, 32, 256], BF, tag="W3", name="W3")
            dma(out=W3.rearrange("p a b c -> p (a b c)"), in_=w3_d[:, :])
            with (tc.tile_pool(name="psum2", bufs=2, space="PSUM") as ps2,
                  tc.tile_pool(name="gates2", bufs=3) as g2):
                for t in range(16):
                    cur, nxt = IN2b[t % 2], IN2b[(t + 1) % 2]
                    nc.vector.tensor_copy(
                        cur[64:96, 1:5, 2:130],
                        XP2[:, t, :].rearrange("p (a b) -> p a b", b=128))
                    ZA = ps2.tile([128, SP2], FP, tag="ZA")
                    ZB = ps2.tile([128, SP2], FP, tag="ZB")
                    for zt, c0 in ((ZA, 0), (ZB, 128)):
                        for off in range(9):
                            dy, dx = off // 3 - 1, off % 3 - 1
                            rhs = cur[:, 1 + dy:5 + dy, 2 + dx:2 + dx + 128]
                            nc.tensor.matmul(zt[:, :], W2[:, off, c0:c0 + 128],
                                             rhs, start=(off == 0),
                                             stop=(off == 8))
                    # ZA rows [f(0:48) - i(64:112) -]; ZB [o(0:48) - g(64:112) -]
                    S2 = g2.tile([128, SP2], BF, tag="S2")
                    SO2 = g2.tile([64, SP2], BF, tag="SO2")
                    nc.scalar.activation(S2[:, :], ZA[:, :], AF.Sigmoid,
                                         bias=B2A[:, 0:1])
                    nc.scalar.activation(SO2[:, :], ZB[0:64, :], AF.Sigmoid,
                                         bias=B2B[0:64, 0:1])
                    nc.scalar.activation(TGC2[64:128, :], ZB[64:128, :],
                                         AF.Tanh, bias=B2B[64:128, 0:1])
                    P22 = g2.tile([128, SP2], BF, tag="P22")
                    nc.vector.tensor_tensor(P22[:, :], S2[:, :], TGC2[:, :],
                                            OP.mult)
                    ZC2 = ps2.tile([64, SP2], FP, tag="ZC2")
                    nc.tensor.matmul(ZC2[:, :], IP2[:, :], P22[:, :],
                                     start=True, stop=True)
                    nc.vector.tensor_copy(TGC2[0:64, :], ZC2[:, :])
                    TC2 = g2.tile([64, SP2], BF, tag="TC2")
                    nc.scalar.activation(TC2[:, :], ZC2[:, :], AF.Tanh)
                    hview = nxt[0:64, 1:5, 2:130]
                    nc.vector.tensor_tensor(
                        hview,
                        SO2[:, :].rearrange("p (a b) -> p a b", b=128),
                        TC2[:, :].rearrange("p (a b) -> p a b", b=128),
                        OP.mult)
                    if t % 2 == 1:
                        k = t // 2
                        PA = g2.tile([64, 4, 128], BF, tag="PA2")
                        nc.vector.tensor_tensor(
                            PA[:, :, :], cur[0:64, 1:5, 2:130],
                            nxt[0:64, 1:5, 2:130], OP.max)
                        PAv = PA.rearrange("p a (b c) -> p a b c", c=2)
                        PX = g2.tile([64, 4, 64], BF, tag="PX2")
                        nc.vector.tensor_tensor(
                            PX[:, :, :], PAv[:, :, :, 0], PAv[:, :, :, 1],
                            OP.max)
                        PXv = PX.rearrange("p (a c) b -> p a c b", c=2)
                        nc.vector.tensor_tensor(
                            PL2R[0:64, 2 * k:2 * k + 2, :],
                            PXv[:, :, 0, :], PXv[:, :, 1, :], OP.max)

            nc.vector.tensor_copy(PL2R[64:128, :, 0:63], PL2R[0:64, :, 1:64])

            # ================================================ conv3/4/5 tail
            with (tc.tile_pool(name="psum3", bufs=1, space="PSUM") as ps3,
                  tc.tile_pool(name="tail", bufs=1) as tl):
                Z3 = ps3.tile([14, 256], FP, tag="Z3")
                nmm = 3 * 32
                i = 0
                for kh in range(3):
                    for j in range(32):
                        nc.tensor.matmul(
                            Z3[:, :], PL2R[:, kh:kh + 14, 2 * j],
                            W3[:, kh, j, :],
                            start=(i == 0), stop=(i == nmm - 1))
                        i += 1
                E0 = tl.tile([14, 256], FP, tag="E0")
                E1 = tl.tile([14, 256], FP, tag="E1")
                E2 = tl.tile([14, 256], FP, tag="E2")
                A3T = tl.tile([14, 256], BF, tag="A3T")
                nc.vector.tensor_tensor(E0[:, :], Z3[:, :], B3R[:, :], OP.add)
                nc.vector.tensor_scalar(E1[:, :], E0[:, :], 0.0, None, OP.min)
                nc.scalar.activation(E1[:, :], E1[:, :], AF.Exp)
                nc.vector.tensor_scalar(E2[:, :], E0[:, :], 0.0, None, OP.max)
                nc.vector.scalar_tensor_tensor(A3T[:, :], E1[:, :], -1.0,
                                               E2[:, :], OP.add, OP.add)
                A3 = tl.tile([128, 2, 14], BF, tag="A3")
                Z3T = ps3.tile([128, 2, 14], BF, tag="Z3T")
                for g in range(2):
                    nc.tensor.transpose(Z3T[:, g, :],
                                        A3T[:, 128 * g:128 * (g + 1)],
                                        IDTB[:, :])
                    nc.scalar.copy(A3[:, g, :], Z3T[:, g, :])
                W4B = tl.tile([128, 2, 128], BF, tag="W4B")
                nc.vector.tensor_copy(W4B.rearrange("p a b -> p (a b)"),
                                      W4.rearrange("p a b -> p (a b)"))
                Z4 = ps3.tile([128, 14], FP, tag="Z4")
                for g in range(2):
                    nc.tensor.matmul(Z4[:, :], W4B[:, g, :], A3[:, g, :],
                                     start=(g == 0), stop=(g == 1))
                F0 = tl.tile([128, 14], FP, tag="F0")
                F1t = tl.tile([128, 14], FP, tag="F1t")
                F2t = tl.tile([128, 14], FP, tag="F2t")
                A4 = tl.tile([128, 14], FP, tag="A4")
                nc.vector.tensor_scalar(F0[:, :], Z4[:, :], B4[:, 0:1], None,
                                        OP.add)
                nc.vector.tensor_scalar(F1t[:, :], F0[:, :], 0.0, None,
                                        OP.min)
                nc.scalar.activation(F1t[:, :], F1t[:, :], AF.Exp)
                nc.vector.tensor_scalar(F2t[:, :], F0[:, :], 0.0, None,
                                        OP.max)
                nc.vector.scalar_tensor_tensor(A4[:, :], F1t[:, :], -1.0,
                                               F2t[:, :], OP.add, OP.add)
                W5B = tl.tile([128, 88], BF, tag="W5B")
                A4B = tl.tile([128, 14], BF, tag="A4B")
                nc.vector.tensor_copy(W5B[:, :], W5[:, :])
                nc.vector.tensor_copy(A4B[:, :], A4[:, :])
                Z5 = ps3.tile([88, 14], FP, tag="Z5")
                nc.tensor.matmul(Z5[:, :], W5B[:, :], A4B[:, :], start=True,
                                 stop=True)
                OUTS = tl.tile([88, 14], FP, tag="OUTS")
                nc.scalar.activation(OUTS[:, :], Z5[:, :], AF.Identity,
                                     bias=B5[:, 0:1])
                dma(out=out_d[:, :], in_=OUTS[:, :])
            lp_cm.__exit__(None, None, None)

    _split_waits(nc, mybir)
    return nc


def _split_waits(nc, mybir):
    """neuronxcc codegen allows one embedded sync wait per instruction;
    hoist extra waits into standalone EventSemaphore ops just before."""
    nsplit = 0
    for bb in nc.m.functions[0].blocks:
        new = []
        for inst in bb.instructions:
            si = inst.sync_info
            if si is not None and si.on_wait is not None and len(si.on_wait) > 1:
                waits = list(si.on_wait)
                for w in waits[:-1]:
                    nsplit += 1
                    ev = mybir.InstEventSemaphore(
                        name=f"{inst.name}-sw{nsplit}",
                        engine=inst.engine,
                        sync_info=mybir.SyncInfo(on_wait=[w], on_update=[]),
                    )
                    new.append(ev)
                inst.sync_info = mybir.SyncInfo(
                    on_wait=[waits[-1]], on_update=list(si.on_update or []))
            new.append(inst)
        try:
            bb.instructions = new
        except Exception:
            bb.instructions[:] = new
    return nc


def _prep_weights(w1, b1, w2, b2, w3, b3, w4, b4, w5, b5):
    f = np.float32
    # clstm1: gate rows [i f g o] -> [i f o g]; h-part and x-part split
    perm1 = np.concatenate([np.arange(0, 64), np.arange(96, 128),
                            np.arange(64, 96)])
    w1p = w1[perm1].astype(f).copy()
    b1p = b1[perm1].astype(f).copy()
    wh = np.transpose(w1p[:, 1:33], (1, 2, 3, 0)).reshape(32, 9, 128)
    w1r = np.zeros((128, 6, 128), f)
    w1r[:, 0, :] = np.concatenate([wh[:, 0], wh[:, 1], wh[:, 3], wh[:, 4]])
    w1r[0:64, 1, :] = np.concatenate([wh[:, 6], wh[:, 7]])
    w1r[0:32, 2, :] = wh[:, 2]
    w1r[0:32, 3, :] = wh[:, 5]
    w1r[0:32, 4, :] = wh[:, 8]
    w1r = w1r.reshape(128, 6 * 128)
    w1x = np.transpose(w1p[:, 0], (1, 2, 0)).reshape(9, 128)
    # clstm2: ci rows [h2(0:48), pad(48:64), x(64:96)];
    # co groups A=[f(0:48),-,i(64:112),-], B=[o(0:48),-,g(64:112),-]
    bi, bf_, bg, bo = b2[0:48], b2[48:96], b2[96:144], b2[144:192]
    wi, wf, wg, wo = w2[0:48], w2[48:96], w2[96:144], w2[144:192]
    zpad = np.zeros((16, 80, 3, 3), np.float32)
    wA = np.concatenate([wf, zpad, wi, zpad]).astype(f)     # (128, 80, 3, 3)
    wB = np.concatenate([wo, zpad, wg, zpad]).astype(f)
    wAB = np.concatenate([wA, wB])                          # (256, 80, 3, 3)
    # input-channel remap to [h2, pad, x]
    w2p = np.zeros((256, 96, 3, 3), f)
    w2p[:, 0:48] = wAB[:, 32:80]
    w2p[:, 64:96] = wAB[:, 0:32]
    w2r = np.transpose(w2p, (1, 2, 3, 0)).reshape(96, 9 * 256)
    z16 = np.zeros(16, f)
    b2a = np.concatenate([bf_, z16, bi, z16]).astype(f)
    b2b = np.concatenate([bo, z16, bg, z16]).astype(f)
    # conv3: [128=(ci,parity padded), kh, kw-pair j, co]; row block 0:48
    # holds even kw taps, 64:112 the odd ones (PL2R's 64:128 partitions
    # hold the +1-shifted columns)
    tmp = np.transpose(w3.astype(f), (1, 2, 3, 0))          # (48,3,64,256)
    w3r = np.zeros((128, 3, 32, 256), f)
    w3r[0:48] = tmp[:, :, 0::2, :]
    w3r[64:112] = tmp[:, :, 1::2, :]
    w4r = np.transpose(w4[:, :, 0, 0].astype(f).reshape(128, 2, 128),
                       (2, 1, 0))
    w5r = w5[:, :, 0, 0].astype(f).T
    i32 = np.eye(32, dtype=f)
    ip2 = np.zeros((128, 64), f)
    ip2[0:48, 0:48] = np.eye(48, dtype=f)
    ip2[64:112, 0:48] = np.eye(48, dtype=f)
    cpf = np.zeros((128, 368), f)
    cpf[:, 0] = b1p
    cpf[:, 1] = b2a
    cpf[:, 2] = b2b
    cpf[:, 3] = b4.astype(f)
    cpf[0:88, 4] = b5.astype(f)
    cpf[0:14, 8:22] = np.eye(14, dtype=f)
    cpf[0:14, 22:278] = np.tile(b3.astype(f)[None, :], (14, 1))
    cpf[:, 280:368] = w5r
    cpb = np.zeros((128, 256), f)
    cpb[0:9, 0:128] = w1x
    cpb[0:64, 128:160] = np.vstack([i32, i32])
    cpb[:, 160:224] = ip2
    cpb[0:14, 224:238] = np.eye(14, dtype=f)
    return dict(
        w1r=w1r, w2r=w2r, w3r=w3r.reshape(128, 3 * 32 * 256),
        w4r=np.ascontiguousarray(w4r.reshape(128, 2 * 128)),
        cpf=cpf, cpb=cpb,
    )


def _get_runner():
    """Build (once) a cached jitted SPMD dispatcher around _bass_exec_p.

    bass_utils.run_bass_kernel_spmd constructs a fresh closure + jax.jit
    object every call, so each dispatch pays full retrace / XLA compile /
    executable load (~2 s). Building the shard_map'd jit once and caching
    it drops steady-state dispatch to transfer + execute."""
    if "runner" in _CACHE:
        return _CACHE["runner"]
    import jax
    from jax.sharding import Mesh, PartitionSpec, NamedSharding
    from jax.experimental.shard_map import shard_map
    import concourse.mybir as mybir
    from concourse import bass2jax

    if "nc" not in _CACHE:
        _CACHE["nc"] = _build_program()
    nc = _CACHE["nc"]
    assert nc.dbg_addr is None
    part_name = (nc.partition_id_tensor.name
                 if nc.partition_id_tensor is not None else None)

    bass2jax.install_neuronx_cc_hook()

    in_names, out_names, out_avals, zero_shapes = [], [], [], []
    for alloc in nc.m.functions[0].allocations:
        if not isinstance(alloc, mybir.MemoryLocationSet):
            continue
        name = alloc.memorylocations[0].name
        if alloc.kind == "ExternalInput":
            if name != part_name:
                in_names.append(name)
        elif alloc.kind == "ExternalOutput":
            shape = tuple(alloc.tensor_shape)
            dtype = mybir.dt.np(alloc.dtype)
            out_names.append(name)
            out_avals.append(jax.core.ShapedArray(shape, dtype))
            zero_shapes.append((shape, dtype))

    n_params = len(in_names)
    n_outs = len(out_names)
    all_in_names = in_names + out_names
    if part_name is not None:
        all_in_names = all_in_names + [part_name]

    def _body(*args):
        operands = list(args)
        if part_name is not None:
            operands.append(bass2jax.partition_id_tensor())
        outs = bass2jax._bass_exec_p.bind(
            *operands,
            out_avals=tuple(out_avals),
            in_names=tuple(all_in_names),
            out_names=tuple(out_names),
            lowering_input_output_aliases=(),
            sim_require_finite=True,
            sim_require_nnan=True,
            nc=nc,
        )
        return tuple(outs)

    devices = jax.devices()[:N_CORES]
    mesh = Mesh(np.asarray(devices), ("core",))
    spec = PartitionSpec("core")
    fn = jax.jit(
        shard_map(_body, mesh=mesh,
                  in_specs=(spec,) * (n_params + n_outs),
                  out_specs=(spec,) * n_outs, check_rep=False),
        donate_argnums=tuple(range(n_params, n_params + n_outs)),
        keep_unused=True)
    runner = dict(fn=fn, in_names=in_names, zero_shapes=zero_shapes,
                  sharding=NamedSharding(mesh, spec), jax=jax)
    _CACHE["runner"] = runner
    return runner


def _weight_key(ws):
    import zlib
    k = 0
    for a in ws:
        a = np.ascontiguousarray(a)
        k = zlib.crc32(a.view(np.uint8).reshape(-1), k)
        k = zlib.adler32(a.view(np.uint8).reshape(-1), k)
    return k


def kernel(x, w1, b1, w2, b2, w3, b3, w4, b4, w5, b5):
    import ml_dtypes

    bf16 = ml_dtypes.bfloat16
    r = _get_runner()
    jx = r["jax"]

    # Device-cache the (replicated) weights keyed on content; steady-state
    # calls only re-ship the activations.
    wkey = _weight_key((w1, b1, w2, b2, w3, b3, w4, b4, w5, b5))
    if _CACHE.get("wkey") != wkey:
        wd = _prep_weights(w1, b1, w2, b2, w3, b3, w4, b4, w5, b5)
        shared = {
            "w1r": wd["w1r"].astype(bf16), "w2r": wd["w2r"].astype(bf16),
            "w3r": wd["w3r"].astype(bf16), "w4r": wd["w4r"],
            "cpf": wd["cpf"], "cpb": wd["cpb"].astype(bf16),
        }
        wdev = {}
        for name, arr in shared.items():
            ga = np.broadcast_to(arr[None], (N_CORES,) + arr.shape)
            ga = np.ascontiguousarray(ga).reshape(N_CORES * arr.shape[0],
                                                  arr.shape[1])
            wdev[name] = jx.device_put(ga, r["sharding"])
        _CACHE["wdev"] = wdev
        _CACHE["wkey"] = wkey
    wdev = _CACHE["wdev"]

    # per-core zero-padded x planes; the 9-offset im2col happens on-device
    xp = np.zeros((N_CORES, T, 10, 260), bf16)
    xp[:, :, 1:9, 2:258] = x[:, 0].astype(bf16)
    xp_g = xp.reshape(N_CORES * T, 2600)

    args = [xp_g if name == "xpad" else wdev[name] for name in r["in_names"]]
    zeros = [np.zeros((N_CORES * s[0],) + s[1:], dt)
             for (s, dt) in r["zero_shapes"]]
    outs = r["fn"](*args, *zeros)
    out = np.asarray(outs[0]).reshape(N_CORES, 88, 14)
    return out[..., None].astype(np.float32)



# revision 20
# speedup vs baseline: 35.0972x; 1.1300x over previous
"""ConvLSTM net (nn_Net_50354196578736) Trainium2 Bass kernel.

Data-parallel over batch: B=8 -> 1 sample per NeuronCore, 8 cores, no
collectives. Per core:
  clstm1 (T=32, 33->128ch, 3x3 SAME on 8x256) -> maxpool3d 2x2x2
  clstm2 (T=16, 80->192ch, 3x3 SAME on 4x128) -> maxpool3d 2x2x2
  reshape -> conv3 (256,48,3,64) VALID + ELU -> conv4 1x1 + ELU -> conv5 1x1

Conv-as-matmul: channels on partitions, zero-padded spatial planes on the
free dim, fp32 PSUM accumulation over shifted-view matmuls, bf16 datapath.

clstm1 K-stacking: the hidden state h (32ch) is kept in 4 partition
quadrants of the recurrent input buffer - quadrant 0 unshifted plus three
spatially shifted replicas (+1 col, +1 row, +1 row+1 col) built by
background SBUF->SBUF DMAs. Kernel offsets whose spatial deltas match the
replica shifts then stack on the contraction axis, collapsing the 9-offset
3x3 conv to 5 matmul passes: one K=128 (offsets (-1,-1),(-1,0),(0,-1),
(0,0)), one K=64 ((1,-1),(1,0)), three K=32. The x-channel contribution is
host-precomputed as a K=9 im2col and folded in as one more accumulating
matmul. (True tile_position row-tiling was probed and hard-faults when
concurrent row tiles accumulate into one PSUM bank.)

Gate math per step: z rows ordered [i,f,o,g]; one sigmoid scan over
[i,f,o]; tanh(g) straight from PSUM partition-shifted into the [tg; c]
pair tile; one paired tensor_tensor makes [sig(i)*tg; sig(f)*c]; the pair
sum c = m1+m2 runs on the PE via a stacked-identity matmul; tanh(c) lands
partition-shifted next to sig(o) for the h product, which writes the next
step's padded conv input directly.

Partition-alignment rules (verified empirically): ops with a PSUM input
may shift partitions freely; two-SBUF-input tensor_tensor needs equal
input bases (output base free); single-SBUF-input ops shift freely;
TensorCopy/Memset need 32-aligned bases.

_split_waits: this walrus build accepts only one embedded sync wait per
instruction; the pass hoists extra waits into standalone EventSemaphore
ops on the same engine. All DMAs use the single SWDGE queue for the same
reason. Host-side numpy does all weight permutation/padding/packing.
"""

import numpy as np

B, T, H, W = 8, 32, 8, 256
F1, F2, F3, F4, NN = 32, 48, 256, 128, 88
N_CORES = 8

PH1, PW1 = 10, 260   # padded layer1 plane; valid (y,x) at (y+1, x+2)
PH2, PW2 = 6, 132    # padded layer2 plane (4x128 maps)
SP1 = H * W          # 2048
SP2 = 4 * 128        # 512

_CACHE = {}


def _build_program():
    import concourse.bass as bass
    import concourse.mybir as mybir
    from concourse.tile import TileContext

    dt = mybir.dt
    AF = mybir.ActivationFunctionType
    OP = mybir.AluOpType
    BF, FP = dt.bfloat16, dt.float32

    nc = bass.Bass(trn_type="TRN2", target_bir_lowering=True, use_seq_codegen=True)

    xpd_d = nc.dram_tensor("xpad", [T, 2600], BF, kind="ExternalInput")
    w1_d = nc.dram_tensor("w1r", [128, 6 * 128], BF, kind="ExternalInput")
    w2_d = nc.dram_tensor("w2r", [96, 9 * 256], BF, kind="ExternalInput")
    w3_d = nc.dram_tensor("w3r", [128, 3 * 32 * 256], BF, kind="ExternalInput")
    w4_d = nc.dram_tensor("w4r", [128, 2 * 128], FP, kind="ExternalInput")
    cpf_d = nc.dram_tensor("cpf", [128, 368], FP, kind="ExternalInput")
    cpb_d = nc.dram_tensor("cpb", [128, 256], BF, kind="ExternalInput")
    out_d = nc.dram_tensor("out", [88, 14], FP, kind="ExternalOutput")

    with TileContext(nc) as tc:
        with tc.tile_pool(name="persist", bufs=1) as pp:
            W1 = pp.tile([128, 6, 128], BF, tag="W1")
            W2 = pp.tile([96, 9, 256], BF, tag="W2")
            W4 = pp.tile([128, 2, 128], FP, tag="W4")
            CPF = pp.tile([128, 368], FP, tag="CPF")
            CPB = pp.tile([128, 256], BF, tag="CPB")
            B1 = CPF[:, 0:1]
            B2A = CPF[:, 1:2]
            B2B = CPF[:, 2:3]
            B4 = CPF[:, 3:4]
            B5 = CPF[0:88, 4:5]
            IDT = CPF[0:14, 8:22]
            B3R = CPF[0:14, 22:278]
            W5 = CPF[:, 280:368]
            W1X = CPB[0:9, 0:128]
            IP1 = CPB[0:64, 128:160]
            IP2 = CPB[:, 160:224]
            IDTB = CPB[0:14, 224:238]
            INb = [pp.tile([128, PH1, PW1], BF, tag=f"IN{k}", name=f"IN{k}")
                   for k in range(2)]
            IN2b = [pp.tile([96, PH2, PW2], BF, tag=f"IN2{k}", name=f"IN2{k}")
                    for k in range(2)]
            TGC1 = pp.tile([64, SP1], BF, tag="TGC1")    # [tg ; c]
            TGC2 = pp.tile([128, SP2], BF, tag="TGC2")   # [c2,-,tg2,-]
            XP2 = pp.tile([32, 16, 512], BF, tag="XP2")
            PL2R = pp.tile([128, 16, 64], BF, tag="PL2R")

            dma = nc.gpsimd.dma_start
            dma(out=W1.rearrange("p a b -> p (a b)"), in_=w1_d[:, :])
            dma(out=W2.rearrange("p a b -> p (a b)"), in_=w2_d[:, :])
            dma(out=W4.rearrange("p a b -> p (a b)"), in_=w4_d[:, :])
            dma(out=CPF[:, :], in_=cpf_d[:, :])
            dma(out=CPB[:, :], in_=cpb_d[:, :])

            for k in range(2):
                nc.vector.memset(INb[k].rearrange("p a b -> p (a b)"), 0.0)
                nc.vector.memset(IN2b[k].rearrange("p a b -> p (a b)"), 0.0)
            nc.vector.memset(TGC1[:, :], 0.0)
            nc.vector.memset(TGC2[:, :], 0.0)

            # ============================= clstm1, 32 steps x 2 half-planes
            with (tc.tile_pool(name="psum1", bufs=2, space="PSUM") as ps1,
                  tc.tile_pool(name="ximp", bufs=1) as xp,
                  tc.tile_pool(name="gates1", bufs=3) as g1):
                S = g1.tile([128, SP1], BF, tag="S1", bufs=1)
                TC = g1.tile([96, SP1], BF, tag="TC", bufs=1)
                # on-device im2col of x: 9 shifted windows of the padded
                # (10x260) planes land on partitions 0-8 of XIM
                XIM = xp.tile([9, T * 2048], BF, tag="XIM")
                XIMv = XIM.rearrange("p (t h w) -> p t h w", t=T, h=8, w=256)
                vx = xpd_d.rearrange("t (h w) -> t h w", h=10, w=260)
                for off in range(9):
                    dy, dx = off // 3 - 1, off % 3 - 1
                    dma(out=XIMv[off:off + 1, :, :, :],
                        in_=vx[:, 1 + dy:9 + dy,
                               2 + dx:258 + dx].unsqueeze(0))
                XIMq = XIM.rearrange("p (t a) -> p t a", t=T)
                # preheat: absorb init-DMA sem into each engine's clock so
                # steady-state instructions carry <=2 sync waits
                PHP = ps1.tile([2, 4], FP, tag="Z1")
                nc.tensor.matmul(PHP[:, :], CPB[0:9, 0:2], CPB[0:9, 0:4],
                                 start=True, stop=True)
                nc.scalar.copy(S[0:2, 0:2], CPF[0:2, 0:2])
                nc.vector.tensor_copy(TGC1[0:2, 0:2], CPF[0:2, 0:2])
                for t in range(T):
                    cur, nxt = INb[t % 2], INb[(t + 1) % 2]
                    for hf in range(2):
                        hs = slice(1024 * hf, 1024 * (hf + 1))
                        Z = ps1.tile([128, 4, 256], FP, tag="Z1")
                        Zq = Z.rearrange("p a b -> p (a b)")
                        for q in range(2):
                            nc.tensor.matmul(
                                Zq[:, 512 * q:512 * (q + 1)],
                                W1X[:, :],
                                XIMq[:, t, 1024 * hf + 512 * q:
                                     1024 * hf + 512 * (q + 1)],
                                start=True, stop=False)
                        groups = ((0, 128, -1, -1), (1, 64, 1, -1),
                                  (2, 32, -1, 1), (3, 32, 0, 1),
                                  (4, 32, 1, 1))
                        for y in range(4):
                            yy = 4 * hf + y
                            for gi, (slot, K, dy, dx) in enumerate(groups):
                                nc.tensor.matmul(
                                    Z[:, y, :],
                                    W1[0:K, slot, :],
                                    cur[0:K, yy + 1 + dy, 2 + dx:2 + dx + 256],
                                    start=False, stop=(gi == 4))
                        Zf = Z.rearrange("p a b -> p (a b)")
                        nc.scalar.activation(S[0:96, hs], Zf[0:96, :], AF.Sigmoid,
                                             bias=B1[0:96, 0:1])
                        nc.scalar.activation(TGC1[0:32, hs], Zf[96:128, :],
                                             AF.Tanh, bias=B1[96:128, 0:1])
                        P2 = g1.tile([64, 1024], BF, tag="P2", bufs=2)
                        nc.vector.tensor_tensor(P2[:, :], S[0:64, hs],
                                                TGC1[:, hs], OP.mult)
                        ZC = ps1.tile([32, 1024], FP, tag="ZC")
                        for q in range(2):
                            nc.tensor.matmul(ZC[:, 512 * q:512 * (q + 1)],
                                             IP1[:, :],
                                             P2[:, 512 * q:512 * (q + 1)],
                                             start=True, stop=True)
                        nc.vector.tensor_copy(TGC1[32:64, hs], ZC[:, :])
                        nc.scalar.activation(TC[64:96, hs], ZC[:, :], AF.Tanh)
                        hview = nxt[0:32, 1 + 4 * hf:5 + 4 * hf, 2:258]
                        nc.vector.tensor_tensor(
                            hview,
                            S[64:96, hs].rearrange("p (a b) -> p a b", b=256),
                            TC[64:96, hs].rearrange("p (a b) -> p a b", b=256),
                            OP.mult)
                        r0, r1 = 1 + 4 * hf, 5 + 4 * hf
                        dma(out=nxt[32:64, r0:r1, 1:257], in_=hview)
                        dma(out=nxt[64:96, r0 - 1:r1 - 1, 2:258], in_=hview)
                        dma(out=nxt[96:128, r0 - 1:r1 - 1, 1:257], in_=hview)
                    if t % 2 == 1:
                        k = t // 2
                        PA = g1.tile([32, 8, 256], BF, tag="PA", bufs=2)
                        nc.vector.tensor_tensor(
                            PA[:, :, :], cur[0:32, 1:9, 2:258],
                            nxt[0:32, 1:9, 2:258], OP.max)
                        PAv = PA.rearrange("p a (b c) -> p a b c", c=2)
                        PX = g1.tile([32, 8, 128], BF, tag="PX", bufs=2)
                        nc.vector.tensor_tensor(
                            PX[:, :, :], PAv[:, :, :, 0], PAv[:, :, :, 1],
                            OP.max)
                        PXv = PX.rearrange("p (a c) b -> p a c b", c=2)
                        XPv = XP2.rearrange("p a (h w) -> p a h w", w=128)
                        nc.vector.tensor_tensor(
                            XPv[:, k, :, :],
                            PXv[:, :, 0, :], PXv[:, :, 1, :], OP.max)

            # ================================================ clstm2, 16 steps
            lp_cm = tc.tile_pool(name="late", bufs=1)
            lp = lp_cm.__enter__()
            W3 = lp.tile([128, 3, 32, 256], BF, tag="W3", name="W3")
            dma(out=W3.rearrange("p a b c -> p (a b c)"), in_=w3_d[:, :])
            with (tc.tile_pool(name="psum2", bufs=2, space="PSUM") as ps2,
                  tc.tile_pool(name="gates2", bufs=3) as g2):
                for t in range(16):
                    cur, nxt = IN2b[t % 2], IN2b[(t + 1) % 2]
                    nc.vector.tensor_copy(
                        cur[64:96, 1:5, 2:130],
                        XP2[:, t, :].rearrange("p (a b) -> p a b", b=128))
                    ZA = ps2.tile([128, SP2], FP, tag="ZA")
                    ZB = ps2.tile([128, SP2], FP, tag="ZB")
                    for zt, c0 in ((ZA, 0), (ZB, 128)):
                        for off in range(9):
                            dy, dx = off // 3 - 1, off % 3 - 1
                            rhs = cur[:, 1 + dy:5 + dy, 2 + dx:2 + dx + 128]
                            nc.tensor.matmul(zt[:, :], W2[:, off, c0:c0 + 128],
                                             rhs, start=(off == 0),
                                             stop=(off == 8))
                    # ZA rows [f(0:48) - i(64:112) -]; ZB [o(0:48) - g(64:112) -]
                    S2 = g2.tile([128, SP2], BF, tag="S2")
                    SO2 = g2.tile([64, SP2], BF, tag="SO2")
                    nc.scalar.activation(S2[:, :], ZA[:, :], AF.Sigmoid,
                                         bias=B2A[:, 0:1])
                    nc.scalar.activation(SO2[:, :], ZB[0:64, :], AF.Sigmoid,
                                         bias=B2B[0:64, 0:1])
                    nc.scalar.activation(TGC2[64:128, :], ZB[64:128, :],
                                         AF.Tanh, bias=B2B[64:128, 0:1])
                    P22 = g2.tile([128, SP2], BF, tag="P22")
                    nc.vector.tensor_tensor(P22[:, :], S2[:, :], TGC2[:, :],
                                            OP.mult)
                    ZC2 = ps2.tile([64, SP2], FP, tag="ZC2")
                    nc.tensor.matmul(ZC2[:, :], IP2[:, :], P22[:, :],
                                     start=True, stop=True)
                    nc.vector.tensor_copy(TGC2[0:64, :], ZC2[:, :])
                    TC2 = g2.tile([64, SP2], BF, tag="TC2")
                    nc.scalar.activation(TC2[:, :], ZC2[:, :], AF.Tanh)
                    hview = nxt[0:64, 1:5, 2:130]
                    nc.vector.tensor_tensor(
                        hview,
                        SO2[:, :].rearrange("p (a b) -> p a b", b=128),
                        TC2[:, :].rearrange("p (a b) -> p a b", b=128),
                        OP.mult)
                    if t % 2 == 1:
                        k = t // 2
                        PA = g2.tile([64, 4, 128], BF, tag="PA2")
                        nc.vector.tensor_tensor(
                            PA[:, :, :], cur[0:64, 1:5, 2:130],
                            nxt[0:64, 1:5, 2:130], OP.max)
                        PAv = PA.rearrange("p a (b c) -> p a b c", c=2)
                        PX = g2.tile([64, 4, 64], BF, tag="PX2")
                        nc.vector.tensor_tensor(
                            PX[:, :, :], PAv[:, :, :, 0], PAv[:, :, :, 1],
                            OP.max)
                        PXv = PX.rearrange("p (a c) b -> p a c b", c=2)
                        nc.vector.tensor_tensor(
                            PL2R[0:64, 2 * k:2 * k + 2, :],
                            PXv[:, :, 0, :], PXv[:, :, 1, :], OP.max)

            nc.vector.tensor_copy(PL2R[64:128, :, 0:63], PL2R[0:64, :, 1:64])

            # ================================================ conv3/4/5 tail
            with (tc.tile_pool(name="psum3", bufs=1, space="PSUM") as ps3,
                  tc.tile_pool(name="tail", bufs=1) as tl):
                Z3 = ps3.tile([14, 256], FP, tag="Z3")
                nmm = 3 * 32
                i = 0
                for kh in range(3):
                    for j in range(32):
                        nc.tensor.matmul(
                            Z3[:, :], PL2R[:, kh:kh + 14, 2 * j],
                            W3[:, kh, j, :],
                            start=(i == 0), stop=(i == nmm - 1))
                        i += 1
                E0 = tl.tile([14, 256], FP, tag="E0")
                E1 = tl.tile([14, 256], FP, tag="E1")
                E2 = tl.tile([14, 256], FP, tag="E2")
                A3T = tl.tile([14, 256], BF, tag="A3T")
                nc.vector.tensor_tensor(E0[:, :], Z3[:, :], B3R[:, :], OP.add)
                nc.vector.tensor_scalar(E1[:, :], E0[:, :], 0.0, None, OP.min)
                nc.scalar.activation(E1[:, :], E1[:, :], AF.Exp)
                nc.vector.tensor_scalar(E2[:, :], E0[:, :], 0.0, None, OP.max)
                nc.vector.scalar_tensor_tensor(A3T[:, :], E1[:, :], -1.0,
                                               E2[:, :], OP.add, OP.add)
                A3 = tl.tile([128, 2, 14], BF, tag="A3")
                Z3T = ps3.tile([128, 2, 14], BF, tag="Z3T")
                for g in range(2):
                    nc.tensor.transpose(Z3T[:, g, :],
                                        A3T[:, 128 * g:128 * (g + 1)],
                                        IDTB[:, :])
                    nc.scalar.copy(A3[:, g, :], Z3T[:, g, :])
                W4B = tl.tile([128, 2, 128], BF, tag="W4B")
                nc.vector.tensor_copy(W4B.rearrange("p a b -> p (a b)"),
                                      W4.rearrange("p a b -> p (a b)"))
                Z4 = ps3.tile([128, 14], FP, tag="Z4")
                for g in range(2):
                    nc.tensor.matmul(Z4[:, :], W4B[:, g, :], A3[:, g, :],
                                     start=(g == 0), stop=(g == 1))
                F0 = tl.tile([128, 14], FP, tag="F0")
                F1t = tl.tile([128, 14], FP, tag="F1t")
                F2t = tl.tile([128, 14], FP, tag="F2t")
                A4 = tl.tile([128, 14], FP, tag="A4")
                nc.vector.tensor_scalar(F0[:, :], Z4[:, :], B4[:, 0:1], None,
                                        OP.add)
                nc.vector.tensor_scalar(F1t[:, :], F0[:, :], 0.0, None,
                                        OP.min)
                nc.scalar.activation(F1t[:, :], F1t[:, :], AF.Exp)
                nc.vector.tensor_scalar(F2t[:, :], F0[:, :], 0.0, None,
                                        OP.max)
                nc.vector.scalar_tensor_tensor(A4[:, :], F1t[:, :], -1.0,
                                               F2t[:, :], OP.add, OP.add)
                W5B = tl.tile([128, 88], BF, tag="W5B")
                A4B = tl.tile([128, 14], BF, tag="A4B")
                nc.vector.tensor_copy(W5B[:, :], W5[:, :])
                nc.vector.tensor_copy(A4B[:, :], A4[:, :])
                Z5 = ps3.tile([88, 14], FP, tag="Z5")
                nc.tensor.matmul(Z5[:, :], W5B[:, :], A4B[:, :], start=True,
                                 stop=True)
                OUTS = tl.tile([88, 14], FP, tag="OUTS")
                nc.scalar.activation(OUTS[:, :], Z5[:, :], AF.Identity,
                                     bias=B5[:, 0:1])
                dma(out=out_d[:, :], in_=OUTS[:, :])
            lp_cm.__exit__(None, None, None)

    _split_waits(nc, mybir)
    return nc


def _split_waits(nc, mybir):
    """neuronxcc codegen allows one embedded sync wait per instruction;
    hoist extra waits into standalone EventSemaphore ops just before."""
    nsplit = 0
    for bb in nc.m.functions[0].blocks:
        new = []
        for inst in bb.instructions:
            si = inst.sync_info
            if si is not None and si.on_wait is not None and len(si.on_wait) > 1:
                waits = list(si.on_wait)
                for w in waits[:-1]:
                    nsplit += 1
                    ev = mybir.InstEventSemaphore(
                        name=f"{inst.name}-sw{nsplit}",
                        engine=inst.engine,
                        sync_info=mybir.SyncInfo(on_wait=[w], on_update=[]),
                    )
                    new.append(ev)
                inst.sync_info = mybir.SyncInfo(
                    on_wait=[waits[-1]], on_update=list(si.on_update or []))
            new.append(inst)
        try:
            bb.instructions = new
        except Exception:
            bb.instructions[:] = new
    return nc


def _prep_weights(w1, b1, w2, b2, w3, b3, w4, b4, w5, b5):
    f = np.float32
    # clstm1: gate rows [i f g o] -> [i f o g]; h-part and x-part split
    perm1 = np.concatenate([np.arange(0, 64), np.arange(96, 128),
                            np.arange(64, 96)])
    w1p = w1[perm1].astype(f).copy()
    b1p = b1[perm1].astype(f).copy()
    wh = np.transpose(w1p[:, 1:33], (1, 2, 3, 0)).reshape(32, 9, 128)
    w1r = np.zeros((128, 6, 128), f)
    w1r[:, 0, :] = np.concatenate([wh[:, 0], wh[:, 1], wh[:, 3], wh[:, 4]])
    w1r[0:64, 1, :] = np.concatenate([wh[:, 6], wh[:, 7]])
    w1r[0:32, 2, :] = wh[:, 2]
    w1r[0:32, 3, :] = wh[:, 5]
    w1r[0:32, 4, :] = wh[:, 8]
    w1r = w1r.reshape(128, 6 * 128)
    w1x = np.transpose(w1p[:, 0], (1, 2, 0)).reshape(9, 128)
    # clstm2: ci rows [h2(0:48), pad(48:64), x(64:96)];
    # co groups A=[f(0:48),-,i(64:112),-], B=[o(0:48),-,g(64:112),-]
    bi, bf_, bg, bo = b2[0:48], b2[48:96], b2[96:144], b2[144:192]
    wi, wf, wg, wo = w2[0:48], w2[48:96], w2[96:144], w2[144:192]
    zpad = np.zeros((16, 80, 3, 3), np.float32)
    wA = np.concatenate([wf, zpad, wi, zpad]).astype(f)     # (128, 80, 3, 3)
    wB = np.concatenate([wo, zpad, wg, zpad]).astype(f)
    wAB = np.concatenate([wA, wB])                          # (256, 80, 3, 3)
    # input-channel remap to [h2, pad, x]
    w2p = np.zeros((256, 96, 3, 3), f)
    w2p[:, 0:48] = wAB[:, 32:80]
    w2p[:, 64:96] = wAB[:, 0:32]
    w2r = np.transpose(w2p, (1, 2, 3, 0)).reshape(96, 9 * 256)
    z16 = np.zeros(16, f)
    b2a = np.concatenate([bf_, z16, bi, z16]).astype(f)
    b2b = np.concatenate([bo, z16, bg, z16]).astype(f)
    # conv3: [128=(ci,parity padded), kh, kw-pair j, co]; row block 0:48
    # holds even kw taps, 64:112 the odd ones (PL2R's 64:128 partitions
    # hold the +1-shifted columns)
    tmp = np.transpose(w3.astype(f), (1, 2, 3, 0))          # (48,3,64,256)
    w3r = np.zeros((128, 3, 32, 256), f)
    w3r[0:48] = tmp[:, :, 0::2, :]
    w3r[64:112] = tmp[:, :, 1::2, :]
    w4r = np.transpose(w4[:, :, 0, 0].astype(f).reshape(128, 2, 128),
                       (2, 1, 0))
    w5r = w5[:, :, 0, 0].astype(f).T
    i32 = np.eye(32, dtype=f)
    ip2 = np.zeros((128, 64), f)
    ip2[0:48, 0:48] = np.eye(48, dtype=f)
    ip2[64:112, 0:48] = np.eye(48, dtype=f)
    cpf = np.zeros((128, 368), f)
    cpf[:, 0] = b1p
    cpf[:, 1] = b2a
    cpf[:, 2] = b2b
    cpf[:, 3] = b4.astype(f)
    cpf[0:88, 4] = b5.astype(f)
    cpf[0:14, 8:22] = np.eye(14, dtype=f)
    cpf[0:14, 22:278] = np.tile(b3.astype(f)[None, :], (14, 1))
    cpf[:, 280:368] = w5r
    cpb = np.zeros((128, 256), f)
    cpb[0:9, 0:128] = w1x
    cpb[0:64, 128:160] = np.vstack([i32, i32])
    cpb[:, 160:224] = ip2
    cpb[0:14, 224:238] = np.eye(14, dtype=f)
    return dict(
        w1r=w1r, w2r=w2r, w3r=w3r.reshape(128, 3 * 32 * 256),
        w4r=np.ascontiguousarray(w4r.reshape(128, 2 * 128)),
        cpf=cpf, cpb=cpb,
    )


def _get_runner():
    """Build (once) a cached jitted SPMD dispatcher around _bass_exec_p.

    bass_utils.run_bass_kernel_spmd constructs a fresh closure + jax.jit
    object every call, so each dispatch pays full retrace / XLA compile /
    executable load (~2 s). Building the shard_map'd jit once and caching
    it drops steady-state dispatch to transfer + execute."""
    if "runner" in _CACHE:
        return _CACHE["runner"]
    import jax
    from jax.sharding import Mesh, PartitionSpec, NamedSharding
    from jax.experimental.shard_map import shard_map
    import concourse.mybir as mybir
    from concourse import bass2jax

    if "nc" not in _CACHE:
        _CACHE["nc"] = _build_program()
    nc = _CACHE["nc"]
    assert nc.dbg_addr is None
    part_name = (nc.partition_id_tensor.name
                 if nc.partition_id_tensor is not None else None)

    bass2jax.install_neuronx_cc_hook()

    in_names, out_names, out_avals, zero_shapes = [], [], [], []
    for alloc in nc.m.functions[0].allocations:
        if not isinstance(alloc, mybir.MemoryLocationSet):
            continue
        name = alloc.memorylocations[0].name
        if alloc.kind == "ExternalInput":
            if name != part_name:
                in_names.append(name)
        elif alloc.kind == "ExternalOutput":
            shape = tuple(alloc.tensor_shape)
            dtype = mybir.dt.np(alloc.dtype)
            out_names.append(name)
            out_avals.append(jax.core.ShapedArray(shape, dtype))
            zero_shapes.append((shape, dtype))

    n_params = len(in_names)
    n_outs = len(out_names)
    all_in_names = in_names + out_names
    if part_name is not None:
        all_in_names = all_in_names + [part_name]

    def _body(*args):
        operands = list(args)
        if part_name is not None:
            operands.append(bass2jax.partition_id_tensor())
        outs = bass2jax._bass_exec_p.bind(
            *operands,
            out_avals=tuple(out_avals),
            in_names=tuple(all_in_names),
            out_names=tuple(out_names),
            lowering_input_output_aliases=(),
            sim_require_finite=True,
            sim_require_nnan=True,
            nc=nc,
        )
        return tuple(outs)

    devices = jax.devices()[:N_CORES]
    mesh = Mesh(np.asarray(devices), ("core",))
    spec = PartitionSpec("core")
    sm = shard_map(_body, mesh=mesh,
                   in_specs=(spec,) * (n_params + n_outs),
                   out_specs=(spec,) * n_outs, check_rep=False)
    samples = []
    for alloc in nc.m.functions[0].allocations:
        if not isinstance(alloc, mybir.MemoryLocationSet):
            continue
        if (alloc.kind == "ExternalInput"
                and alloc.memorylocations[0].name != part_name):
            shape = tuple(alloc.tensor_shape)
            samples.append(np.zeros((N_CORES * shape[0],) + shape[1:],
                                    mybir.dt.np(alloc.dtype)))
    samples += [np.zeros((N_CORES * s[0],) + s[1:], dt)
                for (s, dt) in zero_shapes]
    fn = bass2jax.fast_dispatch_compile(
        lambda: jax.jit(
            sm, donate_argnums=tuple(range(n_params, n_params + n_outs)),
            keep_unused=True).lower(*samples).compile())
    runner = dict(fn=fn, in_names=in_names, zero_shapes=zero_shapes,
                  sharding=NamedSharding(mesh, spec), jax=jax)
    _CACHE["runner"] = runner
    return runner


def _weight_key(ws):
    import zlib
    k = 0
    for a in ws:
        a = np.ascontiguousarray(a)
        k = zlib.crc32(a.view(np.uint8).reshape(-1), k)
    return k


def _upload_weights(jx, sharding, w1, b1, w2, b2, w3, b3, w4, b4, w5, b5):
    import ml_dtypes
    bf16 = ml_dtypes.bfloat16
    wd = _prep_weights(w1, b1, w2, b2, w3, b3, w4, b4, w5, b5)
    shared = {
        "w1r": wd["w1r"].astype(bf16), "w2r": wd["w2r"].astype(bf16),
        "w3r": wd["w3r"].astype(bf16), "w4r": wd["w4r"],
        "cpf": wd["cpf"], "cpb": wd["cpb"].astype(bf16),
    }
    wdev = {}
    for name, arr in shared.items():
        ga = np.broadcast_to(arr[None], (N_CORES,) + arr.shape)
        ga = np.ascontiguousarray(ga).reshape(N_CORES * arr.shape[0],
                                              arr.shape[1])
        wdev[name] = jx.device_put(ga, sharding)
    return wdev


def kernel(x, w1, b1, w2, b2, w3, b3, w4, b4, w5, b5):
    import ml_dtypes

    bf16 = ml_dtypes.bfloat16
    r = _get_runner()
    jx = r["jax"]
    ws = (w1, b1, w2, b2, w3, b3, w4, b4, w5, b5)

    # per-core zero-padded x planes; the 9-offset im2col happens on-device
    xp = np.zeros((N_CORES, T, 10, 260), bf16)
    xp[:, :, 1:9, 2:258] = x[:, 0].astype(bf16)
    xp_g = xp.reshape(N_CORES * T, 2600)

    def launch(wdev):
        args = [xp_g if n == "xpad" else wdev[n] for n in r["in_names"]]
        zeros = [np.zeros((N_CORES * s[0],) + s[1:], dt)
                 for (s, dt) in r["zero_shapes"]]
        return r["fn"](*args, *zeros)

    # Optimistic dispatch: launch with the device-cached weights, verify the
    # weight content hash while the RPC is in flight, and relaunch with
    # freshly uploaded weights on a mismatch.
    outs = None
    if "wdev" in _CACHE:
        outs = launch(_CACHE["wdev"])
    wkey = _weight_key(ws)
    if _CACHE.get("wkey") != wkey:
        _CACHE["wdev"] = _upload_weights(jx, r["sharding"], *ws)
        _CACHE["wkey"] = wkey
        outs = launch(_CACHE["wdev"])
    out = np.asarray(outs[0]).reshape(N_CORES, 88, 14)
    return out[..., None].astype(np.float32)

